# revision 53
# baseline (speedup 1.0000x reference)
"""Trainium2 Bass kernel for nn_Block_local (dual global/banded-local attention block).

Sharding: pure data-parallel - one batch element per NeuronCore (B=8, 8 cores).

Strategy:
  - Host-side marshaling: x transposed to feature-major bf16; weights quantized
    to fp8e4 in DoubleRow-friendly [128, kc, out] layouts (contraction dims
    zero-padded to multiples of 256).
  - Big matmuls (qkv both branches, AV, global proj, fc1, fc2) run fp8
    DoubleRow (0.5 cycles/row) - 4x PE throughput vs f32r.
  - MLP accuracy recovered via fp8 residual compensation: h is split into
    h8 + hlo8 (exact two-term fp8 representation of 4*LN2(x)), and W1/W2 each
    ship as a (hi, lo) fp8 pair with lo = quantization residual.  The extra
    terms are more cheap DoubleRow matmuls into the same PSUM accumulator.
  - Token-half pipelining: scores/AV -> proj -> LN2 -> MLP per 512-token half,
    so each half's LN2/MLP vector work overlaps the other half's PE stream.
  - Local (banded ks=3) attention on DVE in bf16, dripped into the first
    half's scores window; its tiles are freed before the MLP needs SBUF.
"""
import os
import numpy as np
import ml_dtypes

import concourse.bass as bass
import concourse.bacc as bacc
import concourse.mybir as mybir
import concourse.tile as tile
from concourse.bass_utils import run_bass_kernel_spmd
from concourse.masks import make_identity
from concourse import bass_isa
from contextlib import ExitStack

F32 = mybir.dt.float32
BF16 = mybir.dt.bfloat16
FP8 = mybir.dt.float8e4
AF = mybir.ActivationFunctionType
ALU = mybir.AluOpType
AX = mybir.AxisListType
DR = mybir.MatmulPerfMode.DoubleRow

NP_BF16 = ml_dtypes.bfloat16
NP_FP8 = ml_dtypes.float8_e4m3

B, N, C = 8, 1024, 768
GD = 384          # per-branch feature dim
H, D = 6, 64      # heads, head dim
SCALE = D ** -0.5
HID = 3072
EPS = 1e-6
NH = 2            # token n-halves of 512
NHW = N // NH     # 512
MC = N // 128     # 8 token chunks
CC = C // 128     # 6 feature chunks
GC = GD // 128    # 3 feature chunks per branch
JC = HID // 128   # 24 hidden chunks
WS = 64.0         # attention weight quantization scale
WS1 = 256.0       # fc1 weight scale (h8 carries x4 -> PSUM = 1024 * true)
WS2 = 1024.0      # fc2 weight scale


def _build(flags):
    nc = bacc.Bacc("TRN2", target_bir_lowering=False, debug=False)

    xT_d = nc.dram_tensor("xT", (C, N), BF16, kind="ExternalInput")
    wqk_d = nc.dram_tensor("wqk8", (512, 2 * GD), FP8, kind="ExternalInput")
    wv_d = nc.dram_tensor("wv8", (512, GD), FP8, kind="ExternalInput")
    wl_d = nc.dram_tensor("wl8", (512, 3 * GD), FP8, kind="ExternalInput")
    wpg_d = nc.dram_tensor("wpg8", (512, GD), FP8, kind="ExternalInput")
    wpl_d = nc.dram_tensor("wpl8", (512, GD), FP8, kind="ExternalInput")
    w1h_d = nc.dram_tensor("w1h8", (C, HID), FP8, kind="ExternalInput")
    w1l_d = nc.dram_tensor("w1l8", (C, HID), FP8, kind="ExternalInput")
    w2h_d = nc.dram_tensor("w2h8", (HID, C), FP8, kind="ExternalInput")
    w2l_d = nc.dram_tensor("w2l8", (HID, C), FP8, kind="ExternalInput")
    fc1b_d = nc.dram_tensor("fc1bias", (HID,), F32, kind="ExternalInput")
    out_d = nc.dram_tensor("outT", (C, N), F32, kind="ExternalOutput")
    gpb_d = nc.dram_tensor("gpb", (GD,), F32, kind="ExternalInput") if flags["bias_gproj"] else None
    lpb_d = nc.dram_tensor("lpb", (GD,), F32, kind="ExternalInput") if flags["bias_lproj"] else None
    fc2b_d = nc.dram_tensor("fc2bias", (C,), F32, kind="ExternalInput") if flags["bias_fc2"] else None
    g1_d = nc.dram_tensor("ln1gb", (2, GD), F32, kind="ExternalInput") if flags["gb1g"] else None
    l1_d = nc.dram_tensor("ln1lgb", (2, GD), F32, kind="ExternalInput") if flags["gb1l"] else None

    with tile.TileContext(nc) as tc, ExitStack() as top:
        consts = top.enter_context(tc.tile_pool(name="consts", bufs=1))
        core = top.enter_context(tc.tile_pool(name="core", bufs=1))
        wpool = top.enter_context(tc.tile_pool(name="wpool", bufs=1))

        ident16 = consts.tile([128, 128], BF16, tag="ident16")
        make_identity(nc, ident16)
        ones_c = consts.tile([128, 1], BF16, tag="ones_c")
        nc.vector.memset(ones_c, 1.0)
        onebc = consts.tile([1, 128], BF16, tag="onebc")
        nc.vector.memset(onebc, 1.0)
        c64 = consts.tile([1, 64], BF16, tag="c64")
        nc.vector.memset(c64, 1.0 / WS)
        eps_t = consts.tile([1, 1], F32, tag="eps")
        nc.vector.memset(eps_t, EPS)
        eps16_t = consts.tile([1, 1], F32, tag="eps16")
        nc.vector.memset(eps16_t, EPS / 16.0)
        eps16_b = consts.tile([128, 1], F32, tag="eps16b")
        nc.vector.memset(eps16_b, EPS / 16.0)
        eps_b = consts.tile([128, 1], F32, tag="epsb")
        nc.vector.memset(eps_b, EPS)
        zeros16 = consts.tile([1, GD], BF16, tag="zeros16")
        nc.vector.memset(zeros16, 0.0)

        # ---- load inputs ----
        xt = core.tile([128, CC, N], BF16, tag="xt")
        nc.sync.dma_start(xt, xT_d.rearrange("(c p) n -> p c n", p=128))
        wpg = wpool.tile([128, 4, GD], FP8, tag="wpg")
        nc.sync.dma_start(wpg, wpg_d.rearrange("(c p) o -> p c o", p=128))
        wpl = wpool.tile([128, 4, GD], FP8, tag="wpl")
        nc.sync.dma_start(wpl, wpl_d.rearrange("(c p) o -> p c o", p=128))
        w1h = wpool.tile([128, CC, HID], FP8, tag="w1h")
        nc.sync.dma_start(w1h, w1h_d.rearrange("(c p) o -> p c o", p=128))
        w2h = wpool.tile([128, JC, C], FP8, tag="w2h")
        nc.sync.dma_start(w2h, w2h_d.rearrange("(c p) o -> p c o", p=128))
        w1l = wpool.tile([128, CC, HID], FP8, tag="w1l")
        nc.sync.dma_start(w1l, w1l_d.rearrange("(c p) o -> p c o", p=128))
        w2l = wpool.tile([128, JC, C], FP8, tag="w2l")
        nc.sync.dma_start(w2l, w2l_d.rearrange("(c p) o -> p c o", p=128))
        fc1b = wpool.tile([128, JC], F32, tag="fc1b")
        nc.sync.dma_start(fc1b, fc1b_d.rearrange("(c p) -> p c", p=128))

        def load_vec(dram, n_elems, tag):
            t = consts.tile([128, n_elems // 128], F32, tag=tag)
            nc.sync.dma_start(t, dram.rearrange("(c p) -> p c", p=128))
            return t

        gpb = load_vec(gpb_d, GD, "gpb") if gpb_d is not None else None
        lpb = load_vec(lpb_d, GD, "lpb") if lpb_d is not None else None
        fc2b = load_vec(fc2b_d, C, "fc2b") if fc2b_d is not None else None
        g1gb = None
        if g1_d is not None:
            g1gb = consts.tile([128, 2, GC], F32, tag="g1gb")
            nc.sync.dma_start(g1gb, g1_d.rearrange("t (c p) -> p t c", p=128))
        l1gb = None
        if l1_d is not None:
            l1gb = consts.tile([128, 2, GC], F32, tag="l1gb")
            nc.sync.dma_start(l1gb, l1_d.rearrange("t (c p) -> p t c", p=128))

        # persistent attention-lifetime tiles
        attn_scope = tc.tile_pool(name="attn", bufs=1)
        attn = attn_scope.__enter__()
        qkT = attn.tile([128, CC, N], BF16, tag="qkT")     # q chunks 0-2, k 3-5
        vpad8 = attn.tile([128, MC, H * 2 * D], FP8, tag="vpad8")
        o8T = attn.tile([128, 4, N], FP8, tag="o8T")
        nc.gpsimd.memset(o8T[:, GC, :], 0.0)
        o16Tl = attn.tile([128, GC, N], BF16, tag="o16Tl")

        # local-branch tiles: die once all band math + transposes are emitted
        loc_scope = tc.tile_pool(name="locp", bufs=1)
        locp = loc_scope.__enter__()
        ql = locp.tile([128, MC, GD], BF16, tag="ql")
        kl = locp.tile([128, MC, GD], BF16, tag="kl")
        vl = locp.tile([128, MC, GD], BF16, tag="vl")
        km = locp.tile([128, MC, GD], BF16, tag="km")
        kp = locp.tile([128, MC, GD], BF16, tag="kp")
        vm = locp.tile([128, MC, GD], BF16, tag="vm")
        vp = locp.tile([128, MC, GD], BF16, tag="vp")

        # ---------------- LN1 + qkv front-end ----------------
        x8_scope = tc.tile_pool(name="x8p", bufs=1)
        x8p = x8_scope.__enter__()
        x8g = x8p.tile([128, 4, N], FP8, tag="x8g")
        x8l = x8p.tile([128, 4, N], FP8, tag="x8l")
        wqk = x8p.tile([128, 4, 2 * GD], FP8, tag="wqk")
        nc.sync.dma_start(wqk, wqk_d.rearrange("(c p) o -> p c o", p=128))
        wv = x8p.tile([128, 4, GD], FP8, tag="wv")
        nc.sync.dma_start(wv, wv_d.rearrange("(c p) o -> p c o", p=128))
        wl = x8p.tile([128, 4, 3 * GD], FP8, tag="wl")
        nc.sync.dma_start(wl, wl_d.rearrange("(c p) o -> p c o", p=128))
        nc.gpsimd.memset(x8g[:, GC, :], 0.0)
        nc.gpsimd.memset(x8l[:, GC, :], 0.0)

        vview = vpad8.rearrange("p m (h e) -> p m h e", e=2 * D)

        ln1_ctx = {}

        def ln1_stats(lo, nh, ar_p, st_p):
            ns = slice(nh * NHW, (nh + 1) * NHW)
            sq = ar_p.tile([128, GC, NHW], BF16, tag="sq")
            nc.vector.tensor_tensor(sq, xt[:, lo:lo + GC, ns],
                                    xt[:, lo:lo + GC, ns], ALU.mult)
            st = st_p.tile([1, 2 * NHW], F32, tag="st")
            for i in range(GC):
                nc.tensor.matmul(st[:, 0:NHW], ones_c, xt[:, lo + i, ns],
                                 start=(i == 0), stop=(i == GC - 1))
            for i in range(GC):
                nc.tensor.matmul(st[:, NHW:2 * NHW], ones_c, sq[:, i, :],
                                 start=(i == 0), stop=(i == GC - 1))
            ln1_ctx[(lo, nh)] = st

        def ln1_tail(lo, nh, dst, gb, ar_p, st_p, bc_p):
            inv = 1.0 / GD
            ns = slice(nh * NHW, (nh + 1) * NHW)
            st = ln1_ctx.pop((lo, nh))
            me = ar_p.tile([1, 2 * NHW], BF16, tag="me")
            nc.vector.tensor_scalar_mul(me, st, inv)
            mean, e2 = me[:, 0:NHW], me[:, NHW:2 * NHW]
            m2 = ar_p.tile([1, NHW], F32, tag="m2")
            nc.vector.tensor_tensor(m2, mean, mean, ALU.mult)
            nc.vector.tensor_tensor(m2, e2, m2, ALU.subtract)
            nc.scalar.activation(m2, m2, AF.Sqrt, bias=eps_t, scale=1.0)
            rstd = ar_p.tile([1, NHW], BF16, tag="rstd")
            with nc.allow_low_precision(reason="bf16 ln scale"):
                nc.vector.reciprocal(rstd, m2)
            mrb = ar_p.tile([1, NHW], BF16, tag="mrb")
            nc.vector.tensor_tensor(mrb, rstd, mean, ALU.mult)
            bc = bc_p.tile([128, 2 * NHW], F32, tag="bc")
            nc.tensor.matmul(bc[:, 0:NHW], onebc, rstd, start=True, stop=True)
            nc.tensor.matmul(bc[:, NHW:2 * NHW], onebc, mrb, start=True, stop=True)
            rb16 = ar_p.tile([128, NHW], BF16, tag="rb16")
            nc.vector.tensor_copy(rb16, bc[:, 0:NHW])
            mrb16 = ar_p.tile([128, NHW], BF16, tag="mrb16")
            nc.vector.tensor_copy(mrb16, bc[:, NHW:2 * NHW])
            for i, c in enumerate(range(lo, lo + GC)):
                t16 = ar_p.tile([128, NHW], BF16, tag="tq")
                nc.vector.tensor_tensor(t16, xt[:, c, ns], rb16, ALU.mult)
                if gb is not None:
                    t2f = ar_p.tile([128, NHW], F32, tag="t2f")
                    nc.vector.tensor_tensor(t2f, t16, mrb16, ALU.subtract)
                    nc.vector.tensor_scalar(dst[:, i, ns], t2f,
                                            gb[:, 0, i:i + 1], gb[:, 1, i:i + 1],
                                            ALU.mult, ALU.add)
                else:
                    nc.vector.tensor_tensor(dst[:, i, ns], t16, mrb16, ALU.subtract)

        with tc.tile_pool(name="lnar", bufs=1) as lnar_p, \
             tc.tile_pool(name="st0", bufs=2, space="PSUM") as st0_p, \
             tc.tile_pool(name="bc0", bufs=1, space="PSUM") as bc0_p, \
             tc.tile_pool(name="pq0", bufs=2, space="PSUM") as pq0_p:

            def emit_qkT(mo, nh):
                ns = slice(nh * NHW, (nh + 1) * NHW)
                ps = pq0_p.tile([128, NHW], F32, tag="pq0")
                for t in range(2):
                    nc.tensor.matmul(ps, wqk[:, 2 * t:2 * t + 2, mo * 128:(mo + 1) * 128],
                                     x8g[:, 2 * t:2 * t + 2, ns],
                                     start=(t == 0), stop=(t == 1), perf_mode=DR)
                nc.scalar.copy(qkT[:, mo, ns], ps)

            def emit_v(m):
                ms = slice(m * 128, (m + 1) * 128)
                ps = pq0_p.tile([128, NHW], F32, tag="pq0")
                psv = ps[:, 0:GD]
                for t in range(2):
                    nc.tensor.matmul(psv, x8g[:, 2 * t:2 * t + 2, ms],
                                     wv[:, 2 * t:2 * t + 2, :],
                                     start=(t == 0), stop=(t == 1), perf_mode=DR)
                nc.scalar.copy(vview[:, m, :, 0:D],
                               psv.rearrange("p (h d) -> p h d", d=D))

            def emit_lqkv(m):
                ms = slice(m * 128, (m + 1) * 128)
                for pi in range(3):
                    dst = (ql, kl, vl)[pi]
                    ps_l = pq0_p.tile([128, NHW], F32, tag="pq0", name="lqkv_ps")
                    psd = ps_l[:, 0:GD]
                    for t in range(2):
                        nc.tensor.matmul(psd, x8l[:, 2 * t:2 * t + 2, ms],
                                         wl[:, 2 * t:2 * t + 2, pi * GD:(pi + 1) * GD],
                                         start=(t == 0), stop=(t == 1), perf_mode=DR)
                    if pi == 0:
                        nc.scalar.copy(dst[:, m, :], psd)
                    elif pi == 1:
                        nc.vector.tensor_copy(dst[:, m, :], psd)
                    elif m % 2 == 0:
                        nc.scalar.copy(dst[:, m, :], psd)
                    else:
                        nc.vector.tensor_copy(dst[:, m, :], psd)

            for m in range(MC):
                nc.gpsimd.memset(vview[:, m, :, D:2 * D], 1.0)
            ln1_stats(0, 0, lnar_p, st0_p)
            ln1_stats(GC, 0, lnar_p, st0_p)
            ln1_tail(0, 0, x8g, g1gb, lnar_p, st0_p, bc0_p)
            ln1_stats(0, 1, lnar_p, st0_p)
            ln1_tail(GC, 0, x8l, l1gb, lnar_p, st0_p, bc0_p)
            emit_qkT(3, 0)
            emit_qkT(0, 0)
            for m in range(4):
                emit_lqkv(m)
            ln1_stats(GC, 1, lnar_p, st0_p)
            ln1_tail(0, 1, x8g, g1gb, lnar_p, st0_p, bc0_p)
            emit_qkT(3, 1)
            emit_qkT(0, 1)
            for m in range(4):
                emit_v(m)
            ln1_tail(GC, 1, x8l, l1gb, lnar_p, st0_p, bc0_p)
            for m in range(4, MC):
                emit_lqkv(m)
            for m in range(4, MC):
                emit_v(m)
            for mo in (4, 1, 5, 2):
                for nh in range(NH):
                    emit_qkT(mo, nh)
        x8_scope.__exit__(None, None, None)

        # token-shifted local k/v copies (zero rows at sequence edges)
        for src, dst, d in ((kl, km, -1), (vl, vm, -1), (kl, kp, 1), (vl, vp, 1)):
            if d == -1:
                nc.sync.dma_start(dst[1:128, :, :], src[0:127, :, :])
                nc.sync.dma_start(dst[0:1, 1:MC, :], src[127:128, 0:MC - 1, :])
                nc.sync.dma_start(dst[0:1, 0:1, :], zeros16[0:1, 0:GD])
            else:
                nc.sync.dma_start(dst[0:127, :, :], src[1:128, :, :])
                nc.sync.dma_start(dst[127:128, 0:MC - 1, :], src[0:1, 1:MC, :])
                nc.sync.dma_start(dst[127:128, MC - 1:MC, :], zeros16[0:1, 0:GD])

        lw_scope = tc.tile_pool(name="lwork", bufs=2)
        lw_p = lw_scope.__enter__()
        ol_scope = tc.tile_pool(name="olp", bufs=4)
        ol_p = ol_scope.__enter__()
        ptl_p = None  # bound per scores scope

        def emit_local(m):
            """Band attention for token chunk m (DVE bf16) + transpose."""
            qv = ql[:, m].rearrange("p (h d) -> p h d", d=D)
            ed = lw_p.tile([128, 3, H], BF16, tag="ed")
            for di, kk in enumerate((km, kl, kp)):
                prod = lw_p.tile([128, H, D], BF16, tag="prod")
                nc.vector.tensor_tensor(prod, qv,
                                        kk[:, m].rearrange("p (h d) -> p h d", d=D),
                                        ALU.mult)
                with nc.allow_low_precision(reason="bf16 band logits"):
                    nc.vector.reduce_sum(ed[:, di, :], prod, axis=AX.X)
            ee = lw_p.tile([128, 3, H], BF16, tag="ee")
            nc.scalar.activation(ee, ed, AF.Exp, scale=SCALE / (WS * WS))
            if m == 0:
                nc.vector.memset(ee[0:1, 0, :], 0.0)
            if m == MC - 1:
                nc.sync.dma_start(ee[127:128, 2, :], zeros16[0:1, 0:H])
            ssum = lw_p.tile([128, H], BF16, tag="ssum")
            rr = lw_p.tile([128, H], BF16, tag="rr")
            nc.vector.tensor_tensor(ssum, ee[:, 0, :], ee[:, 1, :], ALU.add)
            nc.vector.tensor_tensor(ssum, ssum, ee[:, 2, :], ALU.add)
            with nc.allow_low_precision(reason="bf16 softmax weights"):
                nc.vector.reciprocal(rr, ssum)
            aw = lw_p.tile([128, 3, H], BF16, tag="aw")
            nc.vector.tensor_tensor(aw, ee, rr[:, None, :].to_broadcast((128, 3, H)),
                                    ALU.mult)
            ov = lw_p.tile([128, H, D], BF16, tag="ov")
            t1 = lw_p.tile([128, H, D], BF16, tag="avt")
            nc.vector.tensor_tensor(ov, vm[:, m].rearrange("p (h d) -> p h d", d=D),
                                    aw[:, 0, :, None].to_broadcast((128, H, D)), ALU.mult)
            nc.vector.tensor_tensor(t1, vl[:, m].rearrange("p (h d) -> p h d", d=D),
                                    aw[:, 1, :, None].to_broadcast((128, H, D)), ALU.mult)
            nc.vector.tensor_tensor(ov, ov, t1, ALU.add)
            nc.vector.tensor_tensor(t1, vp[:, m].rearrange("p (h d) -> p h d", d=D),
                                    aw[:, 2, :, None].to_broadcast((128, H, D)), ALU.mult)
            o16l = ol_p.tile([128, GD], BF16, tag="o16l")
            nc.vector.tensor_tensor(o16l.rearrange("p (h d) -> p h d", d=D),
                                    ov, t1, ALU.add)
            pending_tr.append((m, o16l))

        h8 = core.tile([128, CC, N], FP8, tag="h8")
        hlo8 = core.tile([128, CC, N], FP8, tag="hlo8")

        pending_tr = []

        def flush_transposes():
            for m, o16l in pending_tr:
                pt = ptl_p.tile([128, NHW], F32, tag="pq", name="ptr").bitcast(
                    BF16)[:, 0:GC * 128]
                for c in range(GC):
                    nc.tensor.transpose(pt[:, c * 128:(c + 1) * 128],
                                        o16l[:, c * 128:(c + 1) * 128], ident16)
                nc.scalar.copy(o16Tl[:, 0:GC, m * 128:(m + 1) * 128],
                               pt.rearrange("p (c x) -> p c x", x=128))
            pending_tr.clear()

        def emit_scores(nh, queue, drip, hooks=None, end_hook=None):
            ns = slice(nh * NHW, (nh + 1) * NHW)
            with tc.tile_pool(name="pq", bufs=2, space="PSUM") as pq_p, \
                 tc.tile_pool(name="psc", bufs=2, space="PSUM") as ps_p, \
                 tc.tile_pool(name="po", bufs=2, space="PSUM") as po_p, \
                 tc.tile_pool(name="esb", bufs=3) as e_p, \
                 tc.tile_pool(name="small", bufs=2) as sm2_p:
                nonlocal ptl_p
                ptl_p = pq_p
                for h in range(H):
                    hc, hp = h // 2, (h % 2) * 64
                    po = po_p.tile([128, NHW], F32, tag="po")

                    def emit_av(k, e8k):
                        nc.tensor.matmul(po, vpad8[:, 2 * k:2 * k + 2,
                                                   h * 2 * D:(h + 1) * 2 * D],
                                         e8k, start=(k == 0), stop=(k == MC // 2 - 1),
                                         perf_mode=DR)

                    e8s = []
                    for mp in range(MC // 2):
                        ps = ps_p.tile([128, 2 * NHW], F32, tag="ps")
                        for half in range(2):
                            m = 2 * mp + half
                            nc.tensor.matmul(ps[:, half * NHW:(half + 1) * NHW],
                                             qkT[hp:hp + 64, GC + hc, m * 128:(m + 1) * 128],
                                             qkT[hp:hp + 64, hc, ns], start=True, stop=True)
                        e8 = e_p.tile([128, 2, NHW], FP8, tag="e8")
                        nc.scalar.activation(e8.rearrange("p a b -> p (a b)"), ps,
                                             AF.Exp, scale=SCALE / (WS * WS))
                        e8s.append(e8)
                        if mp >= 2:
                            emit_av(mp - 2, e8s[mp - 2])
                    emit_av(2, e8s[2])
                    emit_av(3, e8s[3])
                    for _ in range(drip[h]):
                        if queue:
                            emit_local(queue.pop(0))
                    if hooks and h in hooks:
                        hooks[h](ps_p)
                    rcp = sm2_p.tile([1, NHW], BF16, tag="rcp")
                    with nc.allow_low_precision(reason="bf16 matmul operand"):
                        nc.vector.reciprocal(rcp, po[D:D + 1, :])
                    pb = pq_p.tile([128, NHW], F32, tag="pq", name="pbbc")[0:64, :]
                    nc.tensor.matmul(pb, c64, rcp, start=True, stop=True)
                    pb_sb = sm2_p.tile([64, NHW], BF16, tag="pbsb")
                    nc.vector.tensor_copy(pb_sb, pb)
                    nc.vector.tensor_tensor(o8T[hp:hp + 64, hc, ns], po[0:D, :], pb_sb,
                                            ALU.mult)

                for m in queue:
                    emit_local(m)
                flush_transposes()

                # global proj + residual (PSUM = WS^2 * attn_out)
                for mo in range(GC):
                    ps = pq_p.tile([128, NHW], F32, tag="pq")
                    for t in range(2):
                        nc.tensor.matmul(ps, wpg[:, 2 * t:2 * t + 2, mo * 128:(mo + 1) * 128],
                                         o8T[:, 2 * t:2 * t + 2, ns],
                                         start=(t == 0), stop=(t == 1), perf_mode=DR)
                    if gpb is not None:
                        nc.scalar.activation(ps, ps, AF.Identity,
                                             bias=gpb[:, mo:mo + 1], scale=1.0 / WS)
                        nc.vector.tensor_tensor(xt[:, mo, ns], xt[:, mo, ns], ps, ALU.add)
                    else:
                        nc.vector.scalar_tensor_tensor(xt[:, mo, ns], ps, 1.0 / WS,
                                                       xt[:, mo, ns], ALU.mult, ALU.add)

                # local proj (bf16 moving, fp8 stationary) + residual
                for mo in range(GC):
                    ps = pq_p.tile([128, NHW], F32, tag="pq")
                    for t in range(GC):
                        nc.tensor.matmul(ps, wpl[:, t, mo * 128:(mo + 1) * 128],
                                         o16Tl[:, t, ns],
                                         start=(t == 0), stop=(t == GC - 1))
                    if lpb is not None:
                        nc.scalar.activation(ps, ps, AF.Identity,
                                             bias=lpb[:, mo:mo + 1], scale=1.0 / (WS * WS))
                        nc.vector.tensor_tensor(xt[:, GC + mo, ns], xt[:, GC + mo, ns],
                                                ps, ALU.add)
                    else:
                        nc.vector.scalar_tensor_tensor(xt[:, GC + mo, ns], ps,
                                                       1.0 / (WS * WS),
                                                       xt[:, GC + mo, ns], ALU.mult, ALU.add)
                if end_hook is not None:
                    end_hook(ps_p)

        ln2_ctx = {}

        def emit_ln2_stats(nh, ar_p, ps_pool):
            """LN2 stats on PE (chained sum matmuls into a scores-pool psum
            tile) + DVE smalls.  Emitted inside the scores scope."""
            ns = slice(nh * NHW, (nh + 1) * NHW)
            inv = 1.0 / C
            sq16 = ar_p.tile([128, CC, NHW], BF16, tag="h16")
            nc.scalar.activation(sq16, xt[:, :, ns], AF.Square)
            st = ps_pool.tile([128, 2 * NHW], F32, tag="ps", name="lnst")[0:1, :]
            for i in range(CC):
                nc.tensor.matmul(st[:, 0:NHW], ones_c, xt[:, i, ns],
                                 start=(i == 0), stop=(i == CC - 1))
            for i in range(CC):
                nc.tensor.matmul(st[:, NHW:2 * NHW], ones_c, sq16[:, i, :],
                                 start=(i == 0), stop=(i == CC - 1))
            me = ar_p.tile([1, 2 * NHW], F32, tag="me")
            nc.vector.tensor_scalar_mul(me, st, inv)
            mean, e2 = me[:, 0:NHW], me[:, NHW:2 * NHW]
            m2 = ar_p.tile([1, NHW], F32, tag="m2")
            nc.vector.tensor_tensor(m2, mean, mean, ALU.mult)
            var = ar_p.tile([1, NHW], F32, tag="var")
            nc.vector.tensor_tensor(var, e2, m2, ALU.subtract)
            ln2_ctx[nh] = (var, mean)

        def emit_ln2_tail(nh, ar_p, ps_pool):
            """Sqrt + bf16 broadcasts + 2x tails + h8/hlo8 split."""
            ns = slice(nh * NHW, (nh + 1) * NHW)
            var, mean = ln2_ctx.pop(nh)
            nc.scalar.activation(var, var, AF.Sqrt, bias=eps16_t, scale=1.0 / 16.0)
            rstd = ar_p.tile([1, NHW], BF16, tag="rstd")
            with nc.allow_low_precision(reason="bf16 ln scale"):
                nc.vector.reciprocal(rstd, var)
            mrb = ar_p.tile([1, NHW], BF16, tag="mrb")
            nc.vector.tensor_tensor(mrb, rstd, mean, ALU.mult)
            bc = ps_pool.tile([128, 2 * NHW], F32, tag="ps", name="lnbc")
            nc.tensor.matmul(bc[:, 0:NHW], onebc, rstd, start=True, stop=True)
            nc.tensor.matmul(bc[:, NHW:2 * NHW], onebc, mrb, start=True, stop=True)
            rb16 = ar_p.tile([128, NHW], BF16, tag="rb16")
            nc.vector.tensor_copy(rb16, bc[:, 0:NHW])
            mrb16 = ar_p.tile([128, NHW], BF16, tag="mrb16")
            nc.vector.tensor_copy(mrb16, bc[:, NHW:2 * NHW])
            h16 = ar_p.tile([128, CC, NHW], BF16, tag="h16")
            for c in range(CC):
                t16 = ar_p.tile([128, NHW], BF16, tag="t16")
                nc.vector.tensor_tensor(t16, xt[:, c, ns], rb16, ALU.mult)
                nc.vector.tensor_tensor(h16[:, c, :], t16, mrb16, ALU.subtract)
            for c in range(CC):
                nc.gpsimd.tensor_copy(out=h8[:, c, ns], in_=h16[:, c, :])
                nc.gpsimd.tensor_tensor(hlo8[:, c, ns], h16[:, c, :],
                                        h8[:, c, ns], ALU.subtract)

        def emit_mlp(nh, gl_p, out_p, pm_p, pz_p):
            ns = slice(nh * NHW, (nh + 1) * NHW)
            gl8 = gl_p.tile([128, JC, NHW], FP8, tag="gl8")
            outT = out_p.tile([128, CC, NHW], F32, tag="outT")
            if True:
                for j in range(JC):
                    pm = pm_p.tile([128, NHW], F32, tag="pm")
                    chains = [(w1h, h8), (w1h, hlo8), (w1l, h8)]
                    for ci, (wmat, act) in enumerate(chains):
                        for t in range(GC):
                            nc.tensor.matmul(pm,
                                             wmat[:, 2 * t:2 * t + 2,
                                                  j * 128:(j + 1) * 128],
                                             act[:, 2 * t:2 * t + 2, ns],
                                             start=(ci == 0 and t == 0),
                                             stop=(ci == 2 and t == GC - 1),
                                             perf_mode=DR)
                    nc.scalar.activation(gl8[:, j, :], pm, AF.Gelu,
                                         bias=fc1b[:, j:j + 1], scale=1.0 / (4.0 * WS1))
            if True:
                for mo in range(CC):
                    zp = pz_p.tile([128, NHW], F32, tag="z")
                    for t2 in range(JC // 2):
                        for wmat, first, last in ((w2h, t2 == 0, False),
                                                  (w2l, False, t2 == JC // 2 - 1)):
                            nc.tensor.matmul(zp,
                                             wmat[:, 2 * t2:2 * t2 + 2,
                                                  mo * 128:(mo + 1) * 128],
                                             gl8[:, 2 * t2:2 * t2 + 2, :],
                                             start=first, stop=last, perf_mode=DR)
                    if fc2b is not None:
                        nc.scalar.activation(zp, zp, AF.Identity,
                                             bias=fc2b[:, mo:mo + 1], scale=1.0 / WS2)
                        nc.vector.tensor_tensor(outT[:, mo, :], xt[:, mo, ns],
                                                zp, ALU.add)
                    else:
                        nc.vector.scalar_tensor_tensor(outT[:, mo, :], zp,
                                                       1.0 / WS2,
                                                       xt[:, mo, ns], ALU.mult, ALU.add)
                    if mo == 2:
                        nc.sync.dma_start(
                            out_d.rearrange("(c p) n -> p c n", p=128)[:, 0:3, ns],
                            outT[:, 0:3, :])
                nc.sync.dma_start(
                    out_d.rearrange("(c p) n -> p c n", p=128)[:, 3:CC, ns],
                    outT[:, 3:CC, :])

        # schedule: scores0 -> [scores1 || LN2-0] -> [MLP0 || LN2-1] -> MLP1
        ar_scope = tc.tile_pool(name="arp", bufs=1)
        ar_p = ar_scope.__enter__()
        emit_scores(0, [0, 1, 2, 3], [0, 0, 1, 1, 1, 1])

        def s1_end(ps):
            emit_ln2_stats(1, ar_p, ps)
            emit_ln2_tail(1, ar_p, ps)

        emit_scores(1, [4, 5, 6, 7], [2, 1, 1, 0, 0, 0],
                    hooks={0: lambda ps: emit_ln2_stats(0, ar_p, ps),
                           3: lambda ps: emit_ln2_tail(0, ar_p, ps)},
                    end_hook=s1_end)
        ar_scope.__exit__(None, None, None)
        ol_scope.__exit__(None, None, None)
        lw_scope.__exit__(None, None, None)
        loc_scope.__exit__(None, None, None)
        with tc.tile_pool(name="gl", bufs=1) as gl_p, \
             tc.tile_pool(name="outp", bufs=1) as out_p, \
             tc.tile_pool(name="pm", bufs=4, space="PSUM") as pm_p, \
             tc.tile_pool(name="pz", bufs=2, space="PSUM") as pz_p:
            emit_mlp(0, gl_p, out_p, pm_p, pz_p)
            emit_mlp(1, gl_p, out_p, pm_p, pz_p)

        attn_scope.__exit__(None, None, None)

    nc.compile()
    return nc


def _prep_weights(inp):
    """Host-side: fold LN2 affine into fc1, quantize weights to fp8 with
    residual-compensation pairs for the MLP."""
    def q8(w, scale):
        return np.clip(w.astype(np.float64) * scale, -240.0, 240.0).astype(NP_FP8)

    def q8_pair(w, scale):
        hi = q8(w, scale)
        resid = w.astype(np.float64) * scale - hi.astype(np.float64)
        lo = np.clip(resid, -240.0, 240.0).astype(NP_FP8)
        return hi, lo

    def pad_rows(w, rows):
        out = np.zeros((rows, w.shape[1]), np.float64)
        out[:w.shape[0]] = w
        return out

    gqkv = inp["g_qkv_w"]
    lqkv = inp["l_qkv_w"]
    fc1_w = inp["fc1_w"] * inp["ln2_g"][:, None]
    fc1_bias = inp["fc1_b"].astype(np.float64) + \
        inp["ln2_b"].astype(np.float64) @ inp["fc1_w"].astype(np.float64)
    w1h, w1l = q8_pair(fc1_w, WS1)
    w2h, w2l = q8_pair(inp["fc2_w"], WS2)
    return {
        "wqk8": q8(pad_rows(gqkv[:, :2 * GD], 512), WS),
        "wv8": q8(pad_rows(gqkv[:, 2 * GD:], 512), WS),
        "wl8": q8(pad_rows(lqkv, 512), WS),
        "wpg8": q8(pad_rows(inp["g_proj_w"], 512), WS),
        "wpl8": q8(pad_rows(inp["l_proj_w"], 512), WS),
        "w1h8": w1h, "w1l8": w1l, "w2h8": w2h, "w2l8": w2l,
        "fc1bias": fc1_bias.astype(np.float32),
    }


_NC_CACHE = {}


def kernel(**inputs):
    inp = {k: np.ascontiguousarray(np.asarray(v), dtype=np.float32) for k, v in inputs.items()}
    wmap = _prep_weights(inp)
    flags = {
        "gb1g": not (np.all(inp["ln1_g"] == 1.0) and np.all(inp["ln1_b"] == 0.0)),
        "gb1l": not (np.all(inp["ln1l_g"] == 1.0) and np.all(inp["ln1l_b"] == 0.0)),
        "bias_gproj": bool(np.any(inp["g_proj_b"] != 0.0)),
        "bias_lproj": bool(np.any(inp["l_proj_b"] != 0.0)),
        "bias_fc1": bool(np.any(wmap["fc1bias"] != 0.0)),
        "bias_fc2": bool(np.any(inp["fc2_b"] != 0.0)),
    }
    key = tuple(sorted(flags.items()))
    nc = _NC_CACHE.get(key)
    if nc is None:
        nc = _build(flags)
        _NC_CACHE[key] = nc

    if flags["bias_gproj"]:
        wmap["gpb"] = inp["g_proj_b"]
    if flags["bias_lproj"]:
        wmap["lpb"] = inp["l_proj_b"]
    if flags["bias_fc2"]:
        wmap["fc2bias"] = inp["fc2_b"]
    if flags["gb1g"]:
        wmap["ln1gb"] = np.stack([inp["ln1_g"], inp["ln1_b"]])
    if flags["gb1l"]:
        wmap["ln1lgb"] = np.stack([inp["ln1l_g"], inp["ln1l_b"]])

    x = inp["x"]
    in_maps = [dict(wmap, xT=np.ascontiguousarray(x[b].T).astype(NP_BF16))
               for b in range(B)]
    trace = os.environ.get("BASS_KERNEL_TRACE", "") == "1"
    res = run_bass_kernel_spmd(nc, in_maps, core_ids=list(range(B)),
                               trace=trace, trace_cores=[0] if trace else None)
    if trace:
        print(f"HW exec time: {res.exec_time_ns} ns")
        if res.instructions_and_trace:
            print("trace path:", res.instructions_and_trace[1])
    return np.stack([np.asarray(res.results[b]["outT"]).astype(np.float32).T
                     for b in range(B)])


# revision 54
# speedup vs baseline: 1.0656x; 1.0656x over previous
"""Trainium2 Bass kernel for nn_Block_local (dual global/banded-local attention block).

Sharding: pure data-parallel - one batch element per NeuronCore (B=8, 8 cores).

Strategy:
  - Host-side marshaling: x transposed to feature-major bf16; weights quantized
    to fp8e4 in DoubleRow-friendly [128, kc, out] layouts (contraction dims
    zero-padded to multiples of 256).
  - Big matmuls (qkv both branches, AV, global proj, fc1, fc2) run fp8
    DoubleRow (0.5 cycles/row) - 4x PE throughput vs f32r.
  - MLP accuracy recovered via fp8 residual compensation: h is split into
    h8 + hlo8 (exact two-term fp8 representation of 4*LN2(x)), and W1/W2 each
    ship as a (hi, lo) fp8 pair with lo = quantization residual.  The extra
    terms are more cheap DoubleRow matmuls into the same PSUM accumulator.
  - Token-half pipelining: scores/AV -> proj -> LN2 -> MLP per 512-token half,
    so each half's LN2/MLP vector work overlaps the other half's PE stream.
  - Local (banded ks=3) attention on DVE in bf16, dripped into the first
    half's scores window; its tiles are freed before the MLP needs SBUF.
"""
import os
import numpy as np
import ml_dtypes

import concourse.bass as bass
import concourse.bacc as bacc
import concourse.mybir as mybir
import concourse.tile as tile
from concourse.bass_utils import run_bass_kernel_spmd
from concourse.masks import make_identity
from concourse import bass_isa
from contextlib import ExitStack

F32 = mybir.dt.float32
BF16 = mybir.dt.bfloat16
FP8 = mybir.dt.float8e4
AF = mybir.ActivationFunctionType
ALU = mybir.AluOpType
AX = mybir.AxisListType
DR = mybir.MatmulPerfMode.DoubleRow

NP_BF16 = ml_dtypes.bfloat16
NP_FP8 = ml_dtypes.float8_e4m3

B, N, C = 8, 1024, 768
GD = 384          # per-branch feature dim
H, D = 6, 64      # heads, head dim
SCALE = D ** -0.5
HID = 3072
EPS = 1e-6
NH = 2            # token n-halves of 512
NHW = N // NH     # 512
MC = N // 128     # 8 token chunks
CC = C // 128     # 6 feature chunks
GC = GD // 128    # 3 feature chunks per branch
JC = HID // 128   # 24 hidden chunks
WS = 64.0         # attention weight quantization scale
WS1 = 256.0       # fc1 weight scale (h8 carries x4 -> PSUM = 1024 * true)
WS2 = 1024.0      # fc2 weight scale


def _build(flags):
    nc = bacc.Bacc("TRN2", target_bir_lowering=False, debug=False)

    xT_d = nc.dram_tensor("xT", (C, N), BF16, kind="ExternalInput")
    wqk_d = nc.dram_tensor("wqk8", (512, 2 * GD), FP8, kind="ExternalInput")
    wv_d = nc.dram_tensor("wv8", (512, GD), FP8, kind="ExternalInput")
    wl_d = nc.dram_tensor("wl8", (512, 3 * GD), FP8, kind="ExternalInput")
    wpg_d = nc.dram_tensor("wpg8", (512, GD), FP8, kind="ExternalInput")
    wpl_d = nc.dram_tensor("wpl8", (512, GD), FP8, kind="ExternalInput")
    w1h_d = nc.dram_tensor("w1h8", (C, HID), FP8, kind="ExternalInput")
    w1l_d = nc.dram_tensor("w1l8", (C, HID), FP8, kind="ExternalInput")
    w2h_d = nc.dram_tensor("w2h8", (HID, C), FP8, kind="ExternalInput")
    w2l_d = nc.dram_tensor("w2l8", (HID, C), FP8, kind="ExternalInput")
    fc1b_d = nc.dram_tensor("fc1bias", (HID,), F32, kind="ExternalInput")
    out_d = nc.dram_tensor("outT", (C, N), F32, kind="ExternalOutput")
    gpb_d = nc.dram_tensor("gpb", (GD,), F32, kind="ExternalInput") if flags["bias_gproj"] else None
    lpb_d = nc.dram_tensor("lpb", (GD,), F32, kind="ExternalInput") if flags["bias_lproj"] else None
    fc2b_d = nc.dram_tensor("fc2bias", (C,), F32, kind="ExternalInput") if flags["bias_fc2"] else None
    g1_d = nc.dram_tensor("ln1gb", (2, GD), F32, kind="ExternalInput") if flags["gb1g"] else None
    l1_d = nc.dram_tensor("ln1lgb", (2, GD), F32, kind="ExternalInput") if flags["gb1l"] else None

    with tile.TileContext(nc) as tc, ExitStack() as top:
        consts = top.enter_context(tc.tile_pool(name="consts", bufs=1))
        core = top.enter_context(tc.tile_pool(name="core", bufs=1))
        wpool = top.enter_context(tc.tile_pool(name="wpool", bufs=1))

        ident16 = consts.tile([128, 128], BF16, tag="ident16")
        make_identity(nc, ident16)
        ones_c = consts.tile([128, 1], BF16, tag="ones_c")
        nc.vector.memset(ones_c, 1.0)
        onebc = consts.tile([1, 128], BF16, tag="onebc")
        nc.vector.memset(onebc, 1.0)
        c64 = consts.tile([1, 64], BF16, tag="c64")
        nc.vector.memset(c64, 1.0 / WS)
        eps_t = consts.tile([1, 1], F32, tag="eps")
        nc.vector.memset(eps_t, EPS)
        eps16_t = consts.tile([1, 1], F32, tag="eps16")
        nc.vector.memset(eps16_t, EPS / 16.0)
        eps16_b = consts.tile([128, 1], F32, tag="eps16b")
        nc.vector.memset(eps16_b, EPS / 16.0)
        eps_b = consts.tile([128, 1], F32, tag="epsb")
        nc.vector.memset(eps_b, EPS)
        zeros16 = consts.tile([1, GD], BF16, tag="zeros16")
        nc.vector.memset(zeros16, 0.0)

        # ---- load inputs ----
        xt = core.tile([128, CC, N], BF16, tag="xt")
        nc.sync.dma_start(xt, xT_d.rearrange("(c p) n -> p c n", p=128))
        wpg = wpool.tile([128, 4, GD], FP8, tag="wpg")
        nc.sync.dma_start(wpg, wpg_d.rearrange("(c p) o -> p c o", p=128))
        wpl = wpool.tile([128, 4, GD], FP8, tag="wpl")
        nc.sync.dma_start(wpl, wpl_d.rearrange("(c p) o -> p c o", p=128))
        w1h = wpool.tile([128, CC, HID], FP8, tag="w1h")
        nc.sync.dma_start(w1h, w1h_d.rearrange("(c p) o -> p c o", p=128))
        w2h = wpool.tile([128, JC, C], FP8, tag="w2h")
        nc.sync.dma_start(w2h, w2h_d.rearrange("(c p) o -> p c o", p=128))
        w1l = wpool.tile([128, CC, HID], FP8, tag="w1l")
        nc.sync.dma_start(w1l, w1l_d.rearrange("(c p) o -> p c o", p=128))
        w2l = wpool.tile([128, JC, C], FP8, tag="w2l")
        nc.sync.dma_start(w2l, w2l_d.rearrange("(c p) o -> p c o", p=128))
        fc1b = wpool.tile([128, JC], F32, tag="fc1b")
        nc.sync.dma_start(fc1b, fc1b_d.rearrange("(c p) -> p c", p=128))

        def load_vec(dram, n_elems, tag):
            t = consts.tile([128, n_elems // 128], F32, tag=tag)
            nc.sync.dma_start(t, dram.rearrange("(c p) -> p c", p=128))
            return t

        gpb = load_vec(gpb_d, GD, "gpb") if gpb_d is not None else None
        lpb = load_vec(lpb_d, GD, "lpb") if lpb_d is not None else None
        fc2b = load_vec(fc2b_d, C, "fc2b") if fc2b_d is not None else None
        g1gb = None
        if g1_d is not None:
            g1gb = consts.tile([128, 2, GC], F32, tag="g1gb")
            nc.sync.dma_start(g1gb, g1_d.rearrange("t (c p) -> p t c", p=128))
        l1gb = None
        if l1_d is not None:
            l1gb = consts.tile([128, 2, GC], F32, tag="l1gb")
            nc.sync.dma_start(l1gb, l1_d.rearrange("t (c p) -> p t c", p=128))

        # persistent attention-lifetime tiles
        attn_scope = tc.tile_pool(name="attn", bufs=1)
        attn = attn_scope.__enter__()
        qkT = attn.tile([128, CC, N], BF16, tag="qkT")     # q chunks 0-2, k 3-5
        vpad8 = attn.tile([128, MC, H * 2 * D], FP8, tag="vpad8")
        o8T = attn.tile([128, 4, N], FP8, tag="o8T")
        nc.gpsimd.memset(o8T[:, GC, :], 0.0)
        o16Tl = attn.tile([128, GC, N], BF16, tag="o16Tl")

        # local-branch tiles: die once all band math + transposes are emitted
        loc_scope = tc.tile_pool(name="locp", bufs=1)
        locp = loc_scope.__enter__()
        ql = locp.tile([128, MC, GD], BF16, tag="ql")
        kl = locp.tile([128, MC, GD], BF16, tag="kl")
        vl = locp.tile([128, MC, GD], BF16, tag="vl")
        km = locp.tile([128, MC, GD], BF16, tag="km")
        kp = locp.tile([128, MC, GD], BF16, tag="kp")
        vm = locp.tile([128, MC, GD], BF16, tag="vm")
        vp = locp.tile([128, MC, GD], BF16, tag="vp")

        # ---------------- LN1 + qkv front-end ----------------
        x8_scope = tc.tile_pool(name="x8p", bufs=1)
        x8p = x8_scope.__enter__()
        x8g = x8p.tile([128, 4, N], FP8, tag="x8g")
        x8l = x8p.tile([128, 4, N], FP8, tag="x8l")
        wqk = x8p.tile([128, 4, 2 * GD], FP8, tag="wqk")
        nc.sync.dma_start(wqk, wqk_d.rearrange("(c p) o -> p c o", p=128))
        wv = x8p.tile([128, 4, GD], FP8, tag="wv")
        nc.sync.dma_start(wv, wv_d.rearrange("(c p) o -> p c o", p=128))
        wl = x8p.tile([128, 4, 3 * GD], FP8, tag="wl")
        nc.sync.dma_start(wl, wl_d.rearrange("(c p) o -> p c o", p=128))
        nc.gpsimd.memset(x8g[:, GC, :], 0.0)
        nc.gpsimd.memset(x8l[:, GC, :], 0.0)

        vview = vpad8.rearrange("p m (h e) -> p m h e", e=2 * D)

        ln1_ctx = {}

        def ln1_stats(lo, nh, ar_p, st_p):
            ns = slice(nh * NHW, (nh + 1) * NHW)
            sq = ar_p.tile([128, GC, NHW], BF16, tag="sq")
            nc.vector.tensor_tensor(sq, xt[:, lo:lo + GC, ns],
                                    xt[:, lo:lo + GC, ns], ALU.mult)
            st = st_p.tile([1, 2 * NHW], F32, tag="st")
            for i in range(GC):
                nc.tensor.matmul(st[:, 0:NHW], ones_c, xt[:, lo + i, ns],
                                 start=(i == 0), stop=(i == GC - 1))
            for i in range(GC):
                nc.tensor.matmul(st[:, NHW:2 * NHW], ones_c, sq[:, i, :],
                                 start=(i == 0), stop=(i == GC - 1))
            ln1_ctx[(lo, nh)] = st

        def ln1_tail(lo, nh, dst, gb, ar_p, st_p, bc_p):
            inv = 1.0 / GD
            ns = slice(nh * NHW, (nh + 1) * NHW)
            st = ln1_ctx.pop((lo, nh))
            me = ar_p.tile([1, 2 * NHW], BF16, tag="me")
            nc.vector.tensor_scalar_mul(me, st, inv)
            mean, e2 = me[:, 0:NHW], me[:, NHW:2 * NHW]
            m2 = ar_p.tile([1, NHW], F32, tag="m2")
            nc.vector.tensor_tensor(m2, mean, mean, ALU.mult)
            nc.vector.tensor_tensor(m2, e2, m2, ALU.subtract)
            nc.scalar.activation(m2, m2, AF.Sqrt, bias=eps_t, scale=1.0)
            rstd = ar_p.tile([1, NHW], BF16, tag="rstd")
            with nc.allow_low_precision(reason="bf16 ln scale"):
                nc.vector.reciprocal(rstd, m2)
            mrb = ar_p.tile([1, NHW], BF16, tag="mrb")
            nc.vector.tensor_tensor(mrb, rstd, mean, ALU.mult)
            bc = bc_p.tile([128, 2 * NHW], F32, tag="bc")
            nc.tensor.matmul(bc[:, 0:NHW], onebc, rstd, start=True, stop=True)
            nc.tensor.matmul(bc[:, NHW:2 * NHW], onebc, mrb, start=True, stop=True)
            rb16 = ar_p.tile([128, NHW], BF16, tag="rb16")
            nc.vector.tensor_copy(rb16, bc[:, 0:NHW])
            mrb16 = ar_p.tile([128, NHW], BF16, tag="mrb16")
            nc.vector.tensor_copy(mrb16, bc[:, NHW:2 * NHW])
            for i, c in enumerate(range(lo, lo + GC)):
                t16 = ar_p.tile([128, NHW], BF16, tag="tq")
                nc.vector.tensor_tensor(t16, xt[:, c, ns], rb16, ALU.mult)
                if gb is not None:
                    t2f = ar_p.tile([128, NHW], F32, tag="t2f")
                    nc.vector.tensor_tensor(t2f, t16, mrb16, ALU.subtract)
                    nc.vector.tensor_scalar(dst[:, i, ns], t2f,
                                            gb[:, 0, i:i + 1], gb[:, 1, i:i + 1],
                                            ALU.mult, ALU.add)
                else:
                    nc.vector.tensor_tensor(dst[:, i, ns], t16, mrb16, ALU.subtract)

        with tc.tile_pool(name="lnar", bufs=1) as lnar_p, \
             tc.tile_pool(name="st0", bufs=2, space="PSUM") as st0_p, \
             tc.tile_pool(name="bc0", bufs=1, space="PSUM") as bc0_p, \
             tc.tile_pool(name="pq0", bufs=2, space="PSUM") as pq0_p:

            def emit_qkT(mo, nh):
                ns = slice(nh * NHW, (nh + 1) * NHW)
                ps = pq0_p.tile([128, NHW], F32, tag="pq0")
                for t in range(2):
                    nc.tensor.matmul(ps, wqk[:, 2 * t:2 * t + 2, mo * 128:(mo + 1) * 128],
                                     x8g[:, 2 * t:2 * t + 2, ns],
                                     start=(t == 0), stop=(t == 1), perf_mode=DR)
                nc.scalar.copy(qkT[:, mo, ns], ps)

            def emit_v(m):
                ms = slice(m * 128, (m + 1) * 128)
                ps = pq0_p.tile([128, NHW], F32, tag="pq0")
                psv = ps[:, 0:GD]
                for t in range(2):
                    nc.tensor.matmul(psv, x8g[:, 2 * t:2 * t + 2, ms],
                                     wv[:, 2 * t:2 * t + 2, :],
                                     start=(t == 0), stop=(t == 1), perf_mode=DR)
                nc.scalar.copy(vview[:, m, :, 0:D],
                               psv.rearrange("p (h d) -> p h d", d=D))

            def emit_lqkv(m):
                ms = slice(m * 128, (m + 1) * 128)
                for pi in range(3):
                    dst = (ql, kl, vl)[pi]
                    ps_l = pq0_p.tile([128, NHW], F32, tag="pq0", name="lqkv_ps")
                    psd = ps_l[:, 0:GD]
                    for t in range(2):
                        nc.tensor.matmul(psd, x8l[:, 2 * t:2 * t + 2, ms],
                                         wl[:, 2 * t:2 * t + 2, pi * GD:(pi + 1) * GD],
                                         start=(t == 0), stop=(t == 1), perf_mode=DR)
                    if pi == 0:
                        nc.scalar.copy(dst[:, m, :], psd)
                    elif pi == 1:
                        nc.vector.tensor_copy(dst[:, m, :], psd)
                    elif m % 2 == 0:
                        nc.scalar.copy(dst[:, m, :], psd)
                    else:
                        nc.vector.tensor_copy(dst[:, m, :], psd)

            for m in range(MC):
                nc.gpsimd.memset(vview[:, m, :, D:2 * D], 1.0)
            ln1_stats(0, 0, lnar_p, st0_p)
            ln1_stats(GC, 0, lnar_p, st0_p)
            ln1_tail(0, 0, x8g, g1gb, lnar_p, st0_p, bc0_p)
            ln1_stats(0, 1, lnar_p, st0_p)
            ln1_tail(GC, 0, x8l, l1gb, lnar_p, st0_p, bc0_p)
            emit_qkT(3, 0)
            emit_qkT(0, 0)
            for m in range(4):
                emit_lqkv(m)
            ln1_stats(GC, 1, lnar_p, st0_p)
            ln1_tail(0, 1, x8g, g1gb, lnar_p, st0_p, bc0_p)
            emit_qkT(3, 1)
            emit_qkT(0, 1)
            for m in range(4):
                emit_v(m)
            ln1_tail(GC, 1, x8l, l1gb, lnar_p, st0_p, bc0_p)
            for m in range(4, MC):
                emit_lqkv(m)
            for m in range(4, MC):
                emit_v(m)
            for mo in (4, 1, 5, 2):
                for nh in range(NH):
                    emit_qkT(mo, nh)
        x8_scope.__exit__(None, None, None)

        # token-shifted local k/v copies (zero rows at sequence edges)
        for src, dst, d in ((kl, km, -1), (vl, vm, -1), (kl, kp, 1), (vl, vp, 1)):
            if d == -1:
                nc.sync.dma_start(dst[1:128, :, :], src[0:127, :, :])
                nc.sync.dma_start(dst[0:1, 1:MC, :], src[127:128, 0:MC - 1, :])
                nc.sync.dma_start(dst[0:1, 0:1, :], zeros16[0:1, 0:GD])
            else:
                nc.sync.dma_start(dst[0:127, :, :], src[1:128, :, :])
                nc.sync.dma_start(dst[127:128, 0:MC - 1, :], src[0:1, 1:MC, :])
                nc.sync.dma_start(dst[127:128, MC - 1:MC, :], zeros16[0:1, 0:GD])

        lw_scope = tc.tile_pool(name="lwork", bufs=2)
        lw_p = lw_scope.__enter__()
        ol_scope = tc.tile_pool(name="olp", bufs=4)
        ol_p = ol_scope.__enter__()
        ptl_p = None  # bound per scores scope

        def emit_local(m):
            """Band attention for token chunk m (DVE bf16) + transpose."""
            qv = ql[:, m].rearrange("p (h d) -> p h d", d=D)
            ed = lw_p.tile([128, 3, H], BF16, tag="ed")
            for di, kk in enumerate((km, kl, kp)):
                prod = lw_p.tile([128, H, D], BF16, tag="prod")
                nc.vector.tensor_tensor(prod, qv,
                                        kk[:, m].rearrange("p (h d) -> p h d", d=D),
                                        ALU.mult)
                with nc.allow_low_precision(reason="bf16 band logits"):
                    nc.vector.reduce_sum(ed[:, di, :], prod, axis=AX.X)
            ee = lw_p.tile([128, 3, H], BF16, tag="ee")
            nc.scalar.activation(ee, ed, AF.Exp, scale=SCALE / (WS * WS))
            if m == 0:
                nc.vector.memset(ee[0:1, 0, :], 0.0)
            if m == MC - 1:
                nc.sync.dma_start(ee[127:128, 2, :], zeros16[0:1, 0:H])
            ssum = lw_p.tile([128, H], BF16, tag="ssum")
            rr = lw_p.tile([128, H], BF16, tag="rr")
            nc.vector.tensor_tensor(ssum, ee[:, 0, :], ee[:, 1, :], ALU.add)
            nc.vector.tensor_tensor(ssum, ssum, ee[:, 2, :], ALU.add)
            with nc.allow_low_precision(reason="bf16 softmax weights"):
                nc.vector.reciprocal(rr, ssum)
            aw = lw_p.tile([128, 3, H], BF16, tag="aw")
            nc.vector.tensor_tensor(aw, ee, rr[:, None, :].to_broadcast((128, 3, H)),
                                    ALU.mult)
            ov = lw_p.tile([128, H, D], BF16, tag="ov")
            t1 = lw_p.tile([128, H, D], BF16, tag="avt")
            nc.vector.tensor_tensor(ov, vm[:, m].rearrange("p (h d) -> p h d", d=D),
                                    aw[:, 0, :, None].to_broadcast((128, H, D)), ALU.mult)
            nc.vector.tensor_tensor(t1, vl[:, m].rearrange("p (h d) -> p h d", d=D),
                                    aw[:, 1, :, None].to_broadcast((128, H, D)), ALU.mult)
            nc.vector.tensor_tensor(ov, ov, t1, ALU.add)
            nc.vector.tensor_tensor(t1, vp[:, m].rearrange("p (h d) -> p h d", d=D),
                                    aw[:, 2, :, None].to_broadcast((128, H, D)), ALU.mult)
            o16l = ol_p.tile([128, GD], BF16, tag="o16l")
            nc.vector.tensor_tensor(o16l.rearrange("p (h d) -> p h d", d=D),
                                    ov, t1, ALU.add)
            pending_tr.append((m, o16l))

        h8 = core.tile([128, CC, N], FP8, tag="h8")
        hlo8 = core.tile([128, CC, N], FP8, tag="hlo8")

        pending_tr = []

        def flush_transposes():
            for m, o16l in pending_tr:
                pt = ptl_p.tile([128, NHW], F32, tag="pq", name="ptr").bitcast(
                    BF16)[:, 0:GC * 128]
                for c in range(GC):
                    nc.tensor.transpose(pt[:, c * 128:(c + 1) * 128],
                                        o16l[:, c * 128:(c + 1) * 128], ident16)
                nc.scalar.copy(o16Tl[:, 0:GC, m * 128:(m + 1) * 128],
                               pt.rearrange("p (c x) -> p c x", x=128))
            pending_tr.clear()

        def emit_scores(nh, queue, drip, hooks=None, end_hook=None):
            ns = slice(nh * NHW, (nh + 1) * NHW)
            with tc.tile_pool(name="pq", bufs=2, space="PSUM") as pq_p, \
                 tc.tile_pool(name="psc", bufs=2, space="PSUM") as ps_p, \
                 tc.tile_pool(name="po", bufs=2, space="PSUM") as po_p, \
                 tc.tile_pool(name="esb", bufs=3) as e_p, \
                 tc.tile_pool(name="small", bufs=2) as sm2_p:
                nonlocal ptl_p
                ptl_p = pq_p
                for h in range(H):
                    hc, hp = h // 2, (h % 2) * 64
                    po = po_p.tile([128, NHW], F32, tag="po")

                    def emit_av(k, e8k):
                        nc.tensor.matmul(po, vpad8[:, 2 * k:2 * k + 2,
                                                   h * 2 * D:(h + 1) * 2 * D],
                                         e8k, start=(k == 0), stop=(k == MC // 2 - 1),
                                         perf_mode=DR)

                    e8s = []
                    for mp in range(MC // 2):
                        ps = ps_p.tile([128, 2 * NHW], F32, tag="ps")
                        for half in range(2):
                            m = 2 * mp + half
                            nc.tensor.matmul(ps[:, half * NHW:(half + 1) * NHW],
                                             qkT[hp:hp + 64, GC + hc, m * 128:(m + 1) * 128],
                                             qkT[hp:hp + 64, hc, ns], start=True, stop=True)
                        e8 = e_p.tile([128, 2, NHW], FP8, tag="e8")
                        nc.scalar.activation(e8.rearrange("p a b -> p (a b)"), ps,
                                             AF.Exp, scale=SCALE / (WS * WS))
                        e8s.append(e8)
                        if mp >= 2:
                            emit_av(mp - 2, e8s[mp - 2])
                    emit_av(2, e8s[2])
                    emit_av(3, e8s[3])
                    for _ in range(drip[h]):
                        if queue:
                            emit_local(queue.pop(0))
                    if hooks and h in hooks:
                        hooks[h](ps_p)
                    rcp = sm2_p.tile([1, NHW], BF16, tag="rcp")
                    with nc.allow_low_precision(reason="bf16 matmul operand"):
                        nc.vector.reciprocal(rcp, po[D:D + 1, :])
                    pb = pq_p.tile([128, NHW], F32, tag="pq", name="pbbc")[0:64, :]
                    nc.tensor.matmul(pb, c64, rcp, start=True, stop=True)
                    pb_sb = sm2_p.tile([64, NHW], BF16, tag="pbsb")
                    nc.vector.tensor_copy(pb_sb, pb)
                    nc.vector.tensor_tensor(o8T[hp:hp + 64, hc, ns], po[0:D, :], pb_sb,
                                            ALU.mult)

                for m in queue:
                    emit_local(m)
                flush_transposes()

                # global proj + residual (PSUM = WS^2 * attn_out)
                for mo in range(GC):
                    ps = pq_p.tile([128, NHW], F32, tag="pq")
                    for t in range(2):
                        nc.tensor.matmul(ps, wpg[:, 2 * t:2 * t + 2, mo * 128:(mo + 1) * 128],
                                         o8T[:, 2 * t:2 * t + 2, ns],
                                         start=(t == 0), stop=(t == 1), perf_mode=DR)
                    if gpb is not None:
                        nc.scalar.activation(ps, ps, AF.Identity,
                                             bias=gpb[:, mo:mo + 1], scale=1.0 / WS)
                        nc.vector.tensor_tensor(xt[:, mo, ns], xt[:, mo, ns], ps, ALU.add)
                    else:
                        nc.vector.scalar_tensor_tensor(xt[:, mo, ns], ps, 1.0 / WS,
                                                       xt[:, mo, ns], ALU.mult, ALU.add)

                # local proj (bf16 moving, fp8 stationary) + residual
                for mo in range(GC):
                    ps = pq_p.tile([128, NHW], F32, tag="pq")
                    for t in range(GC):
                        nc.tensor.matmul(ps, wpl[:, t, mo * 128:(mo + 1) * 128],
                                         o16Tl[:, t, ns],
                                         start=(t == 0), stop=(t == GC - 1))
                    if lpb is not None:
                        nc.scalar.activation(ps, ps, AF.Identity,
                                             bias=lpb[:, mo:mo + 1], scale=1.0 / (WS * WS))
                        nc.vector.tensor_tensor(xt[:, GC + mo, ns], xt[:, GC + mo, ns],
                                                ps, ALU.add)
                    else:
                        nc.vector.scalar_tensor_tensor(xt[:, GC + mo, ns], ps,
                                                       1.0 / (WS * WS),
                                                       xt[:, GC + mo, ns], ALU.mult, ALU.add)
                if end_hook is not None:
                    end_hook(ps_p)

        ln2_ctx = {}

        def emit_ln2_stats(nh, ar_p, ps_pool):
            """LN2 stats on PE (chained sum matmuls into a scores-pool psum
            tile) + DVE smalls.  Emitted inside the scores scope."""
            ns = slice(nh * NHW, (nh + 1) * NHW)
            inv = 1.0 / C
            sq16 = ar_p.tile([128, CC, NHW], BF16, tag="h16")
            nc.scalar.activation(sq16, xt[:, :, ns], AF.Square)
            st = ps_pool.tile([128, 2 * NHW], F32, tag="ps", name="lnst")[0:1, :]
            for i in range(CC):
                nc.tensor.matmul(st[:, 0:NHW], ones_c, xt[:, i, ns],
                                 start=(i == 0), stop=(i == CC - 1))
            for i in range(CC):
                nc.tensor.matmul(st[:, NHW:2 * NHW], ones_c, sq16[:, i, :],
                                 start=(i == 0), stop=(i == CC - 1))
            me = ar_p.tile([1, 2 * NHW], F32, tag="me")
            nc.vector.tensor_scalar_mul(me, st, inv)
            mean, e2 = me[:, 0:NHW], me[:, NHW:2 * NHW]
            m2 = ar_p.tile([1, NHW], F32, tag="m2")
            nc.vector.tensor_tensor(m2, mean, mean, ALU.mult)
            var = ar_p.tile([1, NHW], F32, tag="var")
            nc.vector.tensor_tensor(var, e2, m2, ALU.subtract)
            ln2_ctx[nh] = (var, mean)

        def emit_ln2_tail(nh, ar_p, ps_pool):
            """Sqrt + bf16 broadcasts + 2x tails + h8/hlo8 split."""
            ns = slice(nh * NHW, (nh + 1) * NHW)
            var, mean = ln2_ctx.pop(nh)
            nc.scalar.activation(var, var, AF.Sqrt, bias=eps16_t, scale=1.0 / 16.0)
            rstd = ar_p.tile([1, NHW], BF16, tag="rstd")
            with nc.allow_low_precision(reason="bf16 ln scale"):
                nc.vector.reciprocal(rstd, var)
            mrb = ar_p.tile([1, NHW], BF16, tag="mrb")
            nc.vector.tensor_tensor(mrb, rstd, mean, ALU.mult)
            bc = ps_pool.tile([128, 2 * NHW], F32, tag="ps", name="lnbc")
            nc.tensor.matmul(bc[:, 0:NHW], onebc, rstd, start=True, stop=True)
            nc.tensor.matmul(bc[:, NHW:2 * NHW], onebc, mrb, start=True, stop=True)
            rb16 = ar_p.tile([128, NHW], BF16, tag="rb16")
            nc.vector.tensor_copy(rb16, bc[:, 0:NHW])
            mrb16 = ar_p.tile([128, NHW], BF16, tag="mrb16")
            nc.vector.tensor_copy(mrb16, bc[:, NHW:2 * NHW])
            h16 = ar_p.tile([128, CC, NHW], BF16, tag="h16")
            for c in range(CC):
                t16 = ar_p.tile([128, NHW], BF16, tag="t16")
                nc.vector.tensor_tensor(t16, xt[:, c, ns], rb16, ALU.mult)
                nc.vector.tensor_tensor(h16[:, c, :], t16, mrb16, ALU.subtract)
            for c in range(CC):
                nc.gpsimd.tensor_copy(out=h8[:, c, ns], in_=h16[:, c, :])
                nc.gpsimd.tensor_tensor(hlo8[:, c, ns], h16[:, c, :],
                                        h8[:, c, ns], ALU.subtract)

        def emit_mlp(nh, gl_p, out_p, pm_p, pz_p):
            ns = slice(nh * NHW, (nh + 1) * NHW)
            gl8 = gl_p.tile([128, JC, NHW], FP8, tag="gl8")
            outT = out_p.tile([128, CC, NHW], F32, tag="outT")
            if True:
                for j in range(JC):
                    pm = pm_p.tile([128, NHW], F32, tag="pm")
                    chains = [(w1h, h8), (w1h, hlo8)]
                    for ci, (wmat, act) in enumerate(chains):
                        for t in range(GC):
                            nc.tensor.matmul(pm,
                                             wmat[:, 2 * t:2 * t + 2,
                                                  j * 128:(j + 1) * 128],
                                             act[:, 2 * t:2 * t + 2, ns],
                                             start=(ci == 0 and t == 0),
                                             stop=(ci == 1 and t == GC - 1),
                                             perf_mode=DR)
                    nc.scalar.activation(gl8[:, j, :], pm, AF.Gelu,
                                         bias=fc1b[:, j:j + 1], scale=1.0 / (4.0 * WS1))
            if True:
                for mo in range(CC):
                    zp = pz_p.tile([128, NHW], F32, tag="z")
                    for t2 in range(JC // 2):
                        for wmat, first, last in ((w2h, t2 == 0, False),
                                                  (w2l, False, t2 == JC // 2 - 1)):
                            nc.tensor.matmul(zp,
                                             wmat[:, 2 * t2:2 * t2 + 2,
                                                  mo * 128:(mo + 1) * 128],
                                             gl8[:, 2 * t2:2 * t2 + 2, :],
                                             start=first, stop=last, perf_mode=DR)
                    if fc2b is not None:
                        nc.scalar.activation(zp, zp, AF.Identity,
                                             bias=fc2b[:, mo:mo + 1], scale=1.0 / WS2)
                        nc.vector.tensor_tensor(outT[:, mo, :], xt[:, mo, ns],
                                                zp, ALU.add)
                    else:
                        nc.vector.scalar_tensor_tensor(outT[:, mo, :], zp,
                                                       1.0 / WS2,
                                                       xt[:, mo, ns], ALU.mult, ALU.add)
                    if mo == 2:
                        nc.sync.dma_start(
                            out_d.rearrange("(c p) n -> p c n", p=128)[:, 0:3, ns],
                            outT[:, 0:3, :])
                nc.sync.dma_start(
                    out_d.rearrange("(c p) n -> p c n", p=128)[:, 3:CC, ns],
                    outT[:, 3:CC, :])

        # schedule: scores0 -> [scores1 || LN2-0] -> [MLP0 || LN2-1] -> MLP1
        ar_scope = tc.tile_pool(name="arp", bufs=1)
        ar_p = ar_scope.__enter__()
        emit_scores(0, [0, 1, 2, 3], [0, 0, 1, 1, 1, 1])

        def s1_end(ps):
            emit_ln2_stats(1, ar_p, ps)
            emit_ln2_tail(1, ar_p, ps)

        emit_scores(1, [4, 5, 6, 7], [2, 1, 1, 0, 0, 0],
                    hooks={0: lambda ps: emit_ln2_stats(0, ar_p, ps),
                           3: lambda ps: emit_ln2_tail(0, ar_p, ps)},
                    end_hook=s1_end)
        ar_scope.__exit__(None, None, None)
        ol_scope.__exit__(None, None, None)
        lw_scope.__exit__(None, None, None)
        loc_scope.__exit__(None, None, None)
        with tc.tile_pool(name="gl", bufs=1) as gl_p, \
             tc.tile_pool(name="outp", bufs=1) as out_p, \
             tc.tile_pool(name="pm", bufs=4, space="PSUM") as pm_p, \
             tc.tile_pool(name="pz", bufs=2, space="PSUM") as pz_p:
            emit_mlp(0, gl_p, out_p, pm_p, pz_p)
            emit_mlp(1, gl_p, out_p, pm_p, pz_p)

        attn_scope.__exit__(None, None, None)

    nc.compile()
    return nc


def _prep_weights(inp):
    """Host-side: fold LN2 affine into fc1, quantize weights to fp8 with
    residual-compensation pairs for the MLP."""
    def q8(w, scale):
        return np.clip(w.astype(np.float64) * scale, -240.0, 240.0).astype(NP_FP8)

    def q8_pair(w, scale):
        hi = q8(w, scale)
        resid = w.astype(np.float64) * scale - hi.astype(np.float64)
        lo = np.clip(resid, -240.0, 240.0).astype(NP_FP8)
        return hi, lo

    def pad_rows(w, rows):
        out = np.zeros((rows, w.shape[1]), np.float64)
        out[:w.shape[0]] = w
        return out

    gqkv = inp["g_qkv_w"]
    lqkv = inp["l_qkv_w"]
    fc1_w = inp["fc1_w"] * inp["ln2_g"][:, None]
    fc1_bias = inp["fc1_b"].astype(np.float64) + \
        inp["ln2_b"].astype(np.float64) @ inp["fc1_w"].astype(np.float64)
    w1h, w1l = q8_pair(fc1_w, WS1)
    w2h, w2l = q8_pair(inp["fc2_w"], WS2)
    return {
        "wqk8": q8(pad_rows(gqkv[:, :2 * GD], 512), WS),
        "wv8": q8(pad_rows(gqkv[:, 2 * GD:], 512), WS),
        "wl8": q8(pad_rows(lqkv, 512), WS),
        "wpg8": q8(pad_rows(inp["g_proj_w"], 512), WS),
        "wpl8": q8(pad_rows(inp["l_proj_w"], 512), WS),
        "w1h8": w1h, "w1l8": w1l, "w2h8": w2h, "w2l8": w2l,
        "fc1bias": fc1_bias.astype(np.float32),
    }


_NC_CACHE = {}


def kernel(**inputs):
    inp = {k: np.ascontiguousarray(np.asarray(v), dtype=np.float32) for k, v in inputs.items()}
    wmap = _prep_weights(inp)
    flags = {
        "gb1g": not (np.all(inp["ln1_g"] == 1.0) and np.all(inp["ln1_b"] == 0.0)),
        "gb1l": not (np.all(inp["ln1l_g"] == 1.0) and np.all(inp["ln1l_b"] == 0.0)),
        "bias_gproj": bool(np.any(inp["g_proj_b"] != 0.0)),
        "bias_lproj": bool(np.any(inp["l_proj_b"] != 0.0)),
        "bias_fc1": bool(np.any(wmap["fc1bias"] != 0.0)),
        "bias_fc2": bool(np.any(inp["fc2_b"] != 0.0)),
    }
    key = tuple(sorted(flags.items()))
    nc = _NC_CACHE.get(key)
    if nc is None:
        nc = _build(flags)
        _NC_CACHE[key] = nc

    if flags["bias_gproj"]:
        wmap["gpb"] = inp["g_proj_b"]
    if flags["bias_lproj"]:
        wmap["lpb"] = inp["l_proj_b"]
    if flags["bias_fc2"]:
        wmap["fc2bias"] = inp["fc2_b"]
    if flags["gb1g"]:
        wmap["ln1gb"] = np.stack([inp["ln1_g"], inp["ln1_b"]])
    if flags["gb1l"]:
        wmap["ln1lgb"] = np.stack([inp["ln1l_g"], inp["ln1l_b"]])

    x = inp["x"]
    in_maps = [dict(wmap, xT=np.ascontiguousarray(x[b].T).astype(NP_BF16))
               for b in range(B)]
    trace = os.environ.get("BASS_KERNEL_TRACE", "") == "1"
    res = run_bass_kernel_spmd(nc, in_maps, core_ids=list(range(B)),
                               trace=trace, trace_cores=[0] if trace else None)
    if trace:
        print(f"HW exec time: {res.exec_time_ns} ns")
        if res.instructions_and_trace:
            print("trace path:", res.instructions_and_trace[1])
    return np.stack([np.asarray(res.results[b]["outT"]).astype(np.float32).T
                     for b in range(B)])


# revision 66
# speedup vs baseline: 1.0941x; 1.0267x over previous
"""Trainium2 Bass kernel for nn_Block_local (dual global/banded-local attention block).

Sharding: pure data-parallel - one batch element per NeuronCore (B=8, 8 cores).

Strategy:
  - Host-side marshaling: x transposed to feature-major bf16; weights quantized
    to fp8e4 in DoubleRow-friendly [128, kc, out] layouts (contraction dims
    zero-padded to multiples of 256).
  - Big matmuls (qkv both branches, AV, global proj, fc1, fc2) run fp8
    DoubleRow (0.5 cycles/row) - 4x PE throughput vs f32r.
  - MLP accuracy recovered via fp8 residual compensation: h is split into
    h8 + hlo8 (exact two-term fp8 representation of 4*LN2(x)), and W1/W2 each
    ship as a (hi, lo) fp8 pair with lo = quantization residual.  The extra
    terms are more cheap DoubleRow matmuls into the same PSUM accumulator.
  - Token-half pipelining: scores/AV -> proj -> LN2 -> MLP per 512-token half,
    so each half's LN2/MLP vector work overlaps the other half's PE stream.
  - Local (banded ks=3) attention on DVE in bf16, dripped into the first
    half's scores window; its tiles are freed before the MLP needs SBUF.
"""
import os
import numpy as np
import ml_dtypes

import concourse.bass as bass
import concourse.bacc as bacc
import concourse.mybir as mybir
import concourse.tile as tile
from concourse.bass_utils import run_bass_kernel_spmd
from concourse.masks import make_identity
from concourse import bass_isa
from contextlib import ExitStack

F32 = mybir.dt.float32
BF16 = mybir.dt.bfloat16
FP8 = mybir.dt.float8e4
AF = mybir.ActivationFunctionType
ALU = mybir.AluOpType
AX = mybir.AxisListType
DR = mybir.MatmulPerfMode.DoubleRow

NP_BF16 = ml_dtypes.bfloat16
NP_FP8 = ml_dtypes.float8_e4m3

B, N, C = 8, 1024, 768
GD = 384          # per-branch feature dim
H, D = 6, 64      # heads, head dim
SCALE = D ** -0.5
HID = 3072
EPS = 1e-6
NH = 2            # token n-halves of 512
NHW = N // NH     # 512
MC = N // 128     # 8 token chunks
CC = C // 128     # 6 feature chunks
GC = GD // 128    # 3 feature chunks per branch
JC = HID // 128   # 24 hidden chunks
WS = 64.0         # attention weight quantization scale
WS1 = 256.0       # fc1 weight scale (h8 carries x4 -> PSUM = 1024 * true)
WS2 = 1024.0      # fc2 weight scale


def _build(flags):
    nc = bacc.Bacc("TRN2", target_bir_lowering=False, debug=False)

    xT_d = nc.dram_tensor("xT", (C, N), BF16, kind="ExternalInput")
    wqk_d = nc.dram_tensor("wqk8", (512, 2 * GD), FP8, kind="ExternalInput")
    wv_d = nc.dram_tensor("wv8", (512, GD), FP8, kind="ExternalInput")
    wl_d = nc.dram_tensor("wl8", (512, 3 * GD), FP8, kind="ExternalInput")
    wpg_d = nc.dram_tensor("wpg8", (512, GD), FP8, kind="ExternalInput")
    wpl_d = nc.dram_tensor("wpl8", (512, GD), FP8, kind="ExternalInput")
    w1h_d = nc.dram_tensor("w1h8", (C, HID), FP8, kind="ExternalInput")
    w1l_d = nc.dram_tensor("w1l8", (C, HID), FP8, kind="ExternalInput")
    w2h_d = nc.dram_tensor("w2h8", (HID, C), FP8, kind="ExternalInput")
    w2l_d = nc.dram_tensor("w2l8", (HID, C), FP8, kind="ExternalInput")
    fc1b_d = nc.dram_tensor("fc1bias", (HID,), F32, kind="ExternalInput")
    out_d = nc.dram_tensor("outT", (C, N), F32, kind="ExternalOutput")
    gpb_d = nc.dram_tensor("gpb", (GD,), F32, kind="ExternalInput") if flags["bias_gproj"] else None
    lpb_d = nc.dram_tensor("lpb", (GD,), F32, kind="ExternalInput") if flags["bias_lproj"] else None
    fc2b_d = nc.dram_tensor("fc2bias", (C,), F32, kind="ExternalInput") if flags["bias_fc2"] else None
    g1_d = nc.dram_tensor("ln1gb", (2, GD), F32, kind="ExternalInput") if flags["gb1g"] else None
    l1_d = nc.dram_tensor("ln1lgb", (2, GD), F32, kind="ExternalInput") if flags["gb1l"] else None

    with tile.TileContext(nc) as tc, ExitStack() as top:
        consts = top.enter_context(tc.tile_pool(name="consts", bufs=1))
        core = top.enter_context(tc.tile_pool(name="core", bufs=1))
        wpool = top.enter_context(tc.tile_pool(name="wpool", bufs=1))

        ident16 = consts.tile([128, 128], BF16, tag="ident16")
        make_identity(nc, ident16)
        ones_c = consts.tile([128, 1], BF16, tag="ones_c")
        nc.vector.memset(ones_c, 1.0)
        onebc = consts.tile([1, 128], BF16, tag="onebc")
        nc.vector.memset(onebc, 1.0)
        c64 = consts.tile([1, 64], BF16, tag="c64")
        nc.vector.memset(c64, 1.0 / WS)
        eps_t = consts.tile([1, 1], F32, tag="eps")
        nc.vector.memset(eps_t, EPS)
        eps16_t = consts.tile([1, 1], F32, tag="eps16")
        nc.vector.memset(eps16_t, EPS / 16.0)
        eps16_b = consts.tile([128, 1], F32, tag="eps16b")
        nc.vector.memset(eps16_b, EPS / 16.0)
        eps_b = consts.tile([128, 1], F32, tag="epsb")
        nc.vector.memset(eps_b, EPS)
        zeros16 = consts.tile([1, GD], BF16, tag="zeros16")
        nc.vector.memset(zeros16, 0.0)

        # ---- load inputs ----
        xt = core.tile([128, CC, N], BF16, tag="xt")
        nc.sync.dma_start(xt, xT_d.rearrange("(c p) n -> p c n", p=128))
        wpg = wpool.tile([128, 4, GD], FP8, tag="wpg")
        nc.sync.dma_start(wpg, wpg_d.rearrange("(c p) o -> p c o", p=128))
        wpl = wpool.tile([128, 4, GD], FP8, tag="wpl")
        nc.sync.dma_start(wpl, wpl_d.rearrange("(c p) o -> p c o", p=128))
        w1h = wpool.tile([128, CC, HID], FP8, tag="w1h")
        nc.sync.dma_start(w1h, w1h_d.rearrange("(c p) o -> p c o", p=128))
        w2h = wpool.tile([128, JC, C], FP8, tag="w2h")
        nc.sync.dma_start(w2h, w2h_d.rearrange("(c p) o -> p c o", p=128))
        w1l = wpool.tile([128, CC, HID], FP8, tag="w1l")
        nc.sync.dma_start(w1l, w1l_d.rearrange("(c p) o -> p c o", p=128))
        w2l = wpool.tile([128, JC, C], FP8, tag="w2l")
        nc.sync.dma_start(w2l, w2l_d.rearrange("(c p) o -> p c o", p=128))
        fc1b = wpool.tile([128, JC], F32, tag="fc1b")
        nc.sync.dma_start(fc1b, fc1b_d.rearrange("(c p) -> p c", p=128))

        def load_vec(dram, n_elems, tag):
            t = consts.tile([128, n_elems // 128], F32, tag=tag)
            nc.sync.dma_start(t, dram.rearrange("(c p) -> p c", p=128))
            return t

        gpb = load_vec(gpb_d, GD, "gpb") if gpb_d is not None else None
        lpb = load_vec(lpb_d, GD, "lpb") if lpb_d is not None else None
        fc2b = load_vec(fc2b_d, C, "fc2b") if fc2b_d is not None else None
        g1gb = None
        if g1_d is not None:
            g1gb = consts.tile([128, 2, GC], F32, tag="g1gb")
            nc.sync.dma_start(g1gb, g1_d.rearrange("t (c p) -> p t c", p=128))
        l1gb = None
        if l1_d is not None:
            l1gb = consts.tile([128, 2, GC], F32, tag="l1gb")
            nc.sync.dma_start(l1gb, l1_d.rearrange("t (c p) -> p t c", p=128))

        # persistent attention-lifetime tiles
        attn_scope = tc.tile_pool(name="attn", bufs=1)
        attn = attn_scope.__enter__()
        qkT = attn.tile([128, CC, N], BF16, tag="qkT")     # q chunks 0-2, k 3-5
        vpad8 = attn.tile([128, MC, H * 2 * D], FP8, tag="vpad8")
        o8T = attn.tile([128, 4, N], FP8, tag="o8T")
        nc.gpsimd.memset(o8T[:, GC, :], 0.0)
        o16Tl = attn.tile([128, GC, N], BF16, tag="o16Tl")

        # local-branch tiles: die once all band math + transposes are emitted
        loc_scope = tc.tile_pool(name="locp", bufs=1)
        locp = loc_scope.__enter__()
        ql = locp.tile([128, MC, GD], BF16, tag="ql")
        kl = locp.tile([128, MC, GD], BF16, tag="kl")
        vl = locp.tile([128, MC, GD], BF16, tag="vl")
        km = locp.tile([128, MC, GD], BF16, tag="km")
        kp = locp.tile([128, MC, GD], BF16, tag="kp")
        vm = locp.tile([128, MC, GD], BF16, tag="vm")
        vp = locp.tile([128, MC, GD], BF16, tag="vp")

        # ---------------- LN1 + qkv front-end ----------------
        x8_scope = tc.tile_pool(name="x8p", bufs=1)
        x8p = x8_scope.__enter__()
        x8g = x8p.tile([128, 4, N], FP8, tag="x8g")
        x8l = x8p.tile([128, 4, N], FP8, tag="x8l")
        wqk = x8p.tile([128, 4, 2 * GD], FP8, tag="wqk")
        nc.sync.dma_start(wqk, wqk_d.rearrange("(c p) o -> p c o", p=128))
        wv = x8p.tile([128, 4, GD], FP8, tag="wv")
        nc.sync.dma_start(wv, wv_d.rearrange("(c p) o -> p c o", p=128))
        wl = x8p.tile([128, 4, 3 * GD], FP8, tag="wl")
        nc.sync.dma_start(wl, wl_d.rearrange("(c p) o -> p c o", p=128))
        nc.gpsimd.memset(x8g[:, GC, :], 0.0)
        nc.gpsimd.memset(x8l[:, GC, :], 0.0)

        vview = vpad8.rearrange("p m (h e) -> p m h e", e=2 * D)

        ln1_ctx = {}

        def ln1_stats(lo, nh, ar_p, st_p):
            ns = slice(nh * NHW, (nh + 1) * NHW)
            sq = ar_p.tile([128, GC, NHW], BF16, tag="tq")
            nc.vector.tensor_tensor(sq, xt[:, lo:lo + GC, ns],
                                    xt[:, lo:lo + GC, ns], ALU.mult)
            st = st_p.tile([1, 2 * NHW], F32, tag="st")
            for i in range(GC):
                nc.tensor.matmul(st[:, 0:NHW], ones_c, xt[:, lo + i, ns],
                                 start=(i == 0), stop=(i == GC - 1))
            for i in range(GC):
                nc.tensor.matmul(st[:, NHW:2 * NHW], ones_c, sq[:, i, :],
                                 start=(i == 0), stop=(i == GC - 1))
            ln1_ctx[(lo, nh)] = st

        def ln1_tail(lo, nh, dst, gb, ar_p, st_p, bc_p):
            inv = 1.0 / GD
            ns = slice(nh * NHW, (nh + 1) * NHW)
            st = ln1_ctx.pop((lo, nh))
            me = ar_p.tile([1, 2 * NHW], BF16, tag="me")
            nc.vector.tensor_scalar_mul(me, st, inv)
            mean, e2 = me[:, 0:NHW], me[:, NHW:2 * NHW]
            m2 = ar_p.tile([1, NHW], BF16, tag="m2")
            nc.vector.tensor_tensor(m2, mean, mean, ALU.mult)
            nc.vector.tensor_tensor(m2, e2, m2, ALU.subtract)
            nc.scalar.activation(m2, m2, AF.Sqrt, bias=eps_t, scale=1.0)
            rstd = ar_p.tile([1, NHW], BF16, tag="rstd")
            with nc.allow_low_precision(reason="bf16 ln scale"):
                nc.vector.reciprocal(rstd, m2)
            mrb = ar_p.tile([1, NHW], BF16, tag="mrb")
            nc.vector.tensor_tensor(mrb, rstd, mean, ALU.mult)
            bc = bc_p.tile([128, 2 * NHW], F32, tag="bc")
            nc.tensor.matmul(bc[:, 0:NHW], onebc, rstd, start=True, stop=True)
            nc.tensor.matmul(bc[:, NHW:2 * NHW], onebc, mrb, start=True, stop=True)
            rb16 = ar_p.tile([128, NHW], BF16, tag="rb16")
            nc.vector.tensor_copy(rb16, bc[:, 0:NHW])
            mrb16 = ar_p.tile([128, NHW], BF16, tag="mrb16")
            nc.vector.tensor_copy(mrb16, bc[:, NHW:2 * NHW])
            for i, c in enumerate(range(lo, lo + GC)):
                t16 = ar_p.tile([128, NHW], BF16, tag="tq")
                nc.vector.tensor_tensor(t16, xt[:, c, ns], rb16, ALU.mult)
                if gb is not None:
                    t2f = ar_p.tile([128, NHW], F32, tag="t2f")
                    nc.vector.tensor_tensor(t2f, t16, mrb16, ALU.subtract)
                    nc.vector.tensor_scalar(dst[:, i, ns], t2f,
                                            gb[:, 0, i:i + 1], gb[:, 1, i:i + 1],
                                            ALU.mult, ALU.add)
                else:
                    nc.vector.tensor_tensor(dst[:, i, ns], t16, mrb16, ALU.subtract)

        with tc.tile_pool(name="lnar", bufs=2) as lnar_p, \
             tc.tile_pool(name="st0", bufs=2, space="PSUM") as st0_p, \
             tc.tile_pool(name="bc0", bufs=1, space="PSUM") as bc0_p, \
             tc.tile_pool(name="pq0", bufs=2, space="PSUM") as pq0_p:

            def emit_qkT(mo, nh):
                ns = slice(nh * NHW, (nh + 1) * NHW)
                ps = pq0_p.tile([128, NHW], F32, tag="pq0")
                for t in range(2):
                    nc.tensor.matmul(ps, wqk[:, 2 * t:2 * t + 2, mo * 128:(mo + 1) * 128],
                                     x8g[:, 2 * t:2 * t + 2, ns],
                                     start=(t == 0), stop=(t == 1), perf_mode=DR)
                nc.scalar.copy(qkT[:, mo, ns], ps)

            def emit_v(m):
                ms = slice(m * 128, (m + 1) * 128)
                ps = pq0_p.tile([128, NHW], F32, tag="pq0")
                psv = ps[:, 0:GD]
                for t in range(2):
                    nc.tensor.matmul(psv, x8g[:, 2 * t:2 * t + 2, ms],
                                     wv[:, 2 * t:2 * t + 2, :],
                                     start=(t == 0), stop=(t == 1), perf_mode=DR)
                nc.scalar.copy(vview[:, m, :, 0:D],
                               psv.rearrange("p (h d) -> p h d", d=D))

            def emit_lqkv(m):
                ms = slice(m * 128, (m + 1) * 128)
                for pi in range(3):
                    dst = (ql, kl, vl)[pi]
                    ps_l = pq0_p.tile([128, NHW], F32, tag="pq0", name="lqkv_ps")
                    psd = ps_l[:, 0:GD]
                    for t in range(2):
                        nc.tensor.matmul(psd, x8l[:, 2 * t:2 * t + 2, ms],
                                         wl[:, 2 * t:2 * t + 2, pi * GD:(pi + 1) * GD],
                                         start=(t == 0), stop=(t == 1), perf_mode=DR)
                    if pi == 0:
                        nc.scalar.copy(dst[:, m, :], psd)
                    elif pi == 1:
                        nc.vector.tensor_copy(dst[:, m, :], psd)
                    elif m % 2 == 0:
                        nc.scalar.copy(dst[:, m, :], psd)
                    else:
                        nc.vector.tensor_copy(dst[:, m, :], psd)

            for m in range(MC):
                nc.gpsimd.memset(vview[:, m, :, D:2 * D], 1.0)
            ln1_stats(0, 0, lnar_p, st0_p)
            ln1_stats(GC, 0, lnar_p, st0_p)
            ln1_tail(0, 0, x8g, g1gb, lnar_p, st0_p, bc0_p)
            ln1_stats(0, 1, lnar_p, st0_p)
            ln1_tail(GC, 0, x8l, l1gb, lnar_p, st0_p, bc0_p)
            emit_qkT(3, 0)
            emit_qkT(0, 0)
            for m in range(4):
                emit_lqkv(m)
            ln1_stats(GC, 1, lnar_p, st0_p)
            ln1_tail(0, 1, x8g, g1gb, lnar_p, st0_p, bc0_p)
            emit_qkT(3, 1)
            emit_qkT(0, 1)
            for m in range(4):
                emit_v(m)
            ln1_tail(GC, 1, x8l, l1gb, lnar_p, st0_p, bc0_p)
            for m in range(4, MC):
                emit_lqkv(m)
            for m in range(4, MC):
                emit_v(m)
            for mo in (4, 1, 5, 2):
                for nh in range(NH):
                    emit_qkT(mo, nh)
        x8_scope.__exit__(None, None, None)

        # token-shifted local k/v copies (zero rows at sequence edges)
        for src, dst, d in ((kl, km, -1), (vl, vm, -1), (kl, kp, 1), (vl, vp, 1)):
            if d == -1:
                nc.sync.dma_start(dst[1:128, :, :], src[0:127, :, :])
                nc.sync.dma_start(dst[0:1, 1:MC, :], src[127:128, 0:MC - 1, :])
                nc.sync.dma_start(dst[0:1, 0:1, :], zeros16[0:1, 0:GD])
            else:
                nc.sync.dma_start(dst[0:127, :, :], src[1:128, :, :])
                nc.sync.dma_start(dst[127:128, 0:MC - 1, :], src[0:1, 1:MC, :])
                nc.sync.dma_start(dst[127:128, MC - 1:MC, :], zeros16[0:1, 0:GD])

        lw_scope = tc.tile_pool(name="lwork", bufs=2)
        lw_p = lw_scope.__enter__()
        ol_scope = tc.tile_pool(name="olp", bufs=4)
        ol_p = ol_scope.__enter__()
        ptl_p = None  # bound per scores scope

        def emit_local(m):
            """Band attention for token chunk m (DVE bf16) + transpose."""
            qv = ql[:, m].rearrange("p (h d) -> p h d", d=D)
            ed = lw_p.tile([128, 3, H], BF16, tag="ed")
            for di, kk in enumerate((km, kl, kp)):
                prod = lw_p.tile([128, H, D], BF16, tag="prod")
                nc.vector.tensor_tensor(prod, qv,
                                        kk[:, m].rearrange("p (h d) -> p h d", d=D),
                                        ALU.mult)
                with nc.allow_low_precision(reason="bf16 band logits"):
                    nc.vector.reduce_sum(ed[:, di, :], prod, axis=AX.X)
            ee = lw_p.tile([128, 3, H], BF16, tag="ee")
            nc.scalar.activation(ee, ed, AF.Exp, scale=SCALE / (WS * WS))
            if m == 0:
                nc.vector.memset(ee[0:1, 0, :], 0.0)
            if m == MC - 1:
                nc.sync.dma_start(ee[127:128, 2, :], zeros16[0:1, 0:H])
            ssum = lw_p.tile([128, H], BF16, tag="ssum")
            rr = lw_p.tile([128, H], BF16, tag="rr")
            nc.vector.tensor_tensor(ssum, ee[:, 0, :], ee[:, 1, :], ALU.add)
            nc.vector.tensor_tensor(ssum, ssum, ee[:, 2, :], ALU.add)
            with nc.allow_low_precision(reason="bf16 softmax weights"):
                nc.vector.reciprocal(rr, ssum)
            aw = lw_p.tile([128, 3, H], BF16, tag="aw")
            nc.vector.tensor_tensor(aw, ee, rr[:, None, :].to_broadcast((128, 3, H)),
                                    ALU.mult)
            ov = lw_p.tile([128, H, D], BF16, tag="ov")
            t1 = lw_p.tile([128, H, D], BF16, tag="avt")
            nc.vector.tensor_tensor(ov, vm[:, m].rearrange("p (h d) -> p h d", d=D),
                                    aw[:, 0, :, None].to_broadcast((128, H, D)), ALU.mult)
            nc.vector.tensor_tensor(t1, vl[:, m].rearrange("p (h d) -> p h d", d=D),
                                    aw[:, 1, :, None].to_broadcast((128, H, D)), ALU.mult)
            nc.vector.tensor_tensor(ov, ov, t1, ALU.add)
            nc.vector.tensor_tensor(t1, vp[:, m].rearrange("p (h d) -> p h d", d=D),
                                    aw[:, 2, :, None].to_broadcast((128, H, D)), ALU.mult)
            o16l = ol_p.tile([128, GD], BF16, tag="o16l")
            nc.vector.tensor_tensor(o16l.rearrange("p (h d) -> p h d", d=D),
                                    ov, t1, ALU.add)
            pending_tr.append((m, o16l))

        h8 = core.tile([128, CC, N], FP8, tag="h8")
        hlo8 = core.tile([128, CC, N], FP8, tag="hlo8")

        pending_tr = []

        def flush_transposes():
            for m, o16l in pending_tr:
                pt = ptl_p.tile([128, NHW], F32, tag="pq", name="ptr").bitcast(
                    BF16)[:, 0:GC * 128]
                for c in range(GC):
                    nc.tensor.transpose(pt[:, c * 128:(c + 1) * 128],
                                        o16l[:, c * 128:(c + 1) * 128], ident16)
                nc.scalar.copy(o16Tl[:, 0:GC, m * 128:(m + 1) * 128],
                               pt.rearrange("p (c x) -> p c x", x=128))
            pending_tr.clear()

        def emit_scores(nh, queue, drip, hooks=None, end_hook=None):
            ns = slice(nh * NHW, (nh + 1) * NHW)
            with tc.tile_pool(name="po", bufs=2, space="PSUM") as po_p, \
                 tc.tile_pool(name="psc", bufs=2, space="PSUM") as ps_p, \
                 tc.tile_pool(name="pq", bufs=2, space="PSUM") as pq_p, \
                 tc.tile_pool(name="esb", bufs=4) as e_p, \
                 tc.tile_pool(name="small", bufs=2) as sm2_p:
                nonlocal ptl_p
                ptl_p = pq_p
                for h in range(H):
                    hc, hp = h // 2, (h % 2) * 64
                    po = po_p.tile([128, NHW], F32, tag="po")

                    def emit_av(k, e8k):
                        nc.tensor.matmul(po, vpad8[:, 2 * k:2 * k + 2,
                                                   h * 2 * D:(h + 1) * 2 * D],
                                         e8k, start=(k == 0), stop=(k == MC // 2 - 1),
                                         perf_mode=DR)

                    e8s = []
                    for mp in range(MC // 2):
                        ps = ps_p.tile([128, 2 * NHW], F32, tag="ps")
                        for half in range(2):
                            m = 2 * mp + half
                            nc.tensor.matmul(ps[:, half * NHW:(half + 1) * NHW],
                                             qkT[hp:hp + 64, GC + hc, m * 128:(m + 1) * 128],
                                             qkT[hp:hp + 64, hc, ns], start=True, stop=True)
                        e8 = e_p.tile([128, 2, NHW], FP8, tag="e8")
                        nc.scalar.activation(e8.rearrange("p a b -> p (a b)"), ps,
                                             AF.Exp, scale=SCALE / (WS * WS))
                        e8s.append(e8)
                        if mp >= 2:
                            emit_av(mp - 2, e8s[mp - 2])
                    emit_av(2, e8s[2])
                    emit_av(3, e8s[3])
                    for _ in range(drip[h]):
                        if queue:
                            emit_local(queue.pop(0))
                    if hooks and h in hooks:
                        hooks[h](ps_p)
                    rcp = sm2_p.tile([1, NHW], BF16, tag="rcp")
                    with nc.allow_low_precision(reason="bf16 matmul operand"):
                        nc.vector.reciprocal(rcp, po[D:D + 1, :])
                    pb = pq_p.tile([128, NHW], F32, tag="pq", name="pbbc")[0:64, :]
                    nc.tensor.matmul(pb, c64, rcp, start=True, stop=True)
                    pb_sb = sm2_p.tile([64, NHW], BF16, tag="pbsb")
                    nc.scalar.copy(pb_sb, pb)
                    nc.vector.tensor_tensor(o8T[hp:hp + 64, hc, ns], po[0:D, :], pb_sb,
                                            ALU.mult)

                for m in queue:
                    emit_local(m)
                flush_transposes()

                # global proj + residual (PSUM = WS^2 * attn_out)
                for mo in range(GC):
                    ps = pq_p.tile([128, NHW], F32, tag="pq")
                    for t in range(2):
                        nc.tensor.matmul(ps, wpg[:, 2 * t:2 * t + 2, mo * 128:(mo + 1) * 128],
                                         o8T[:, 2 * t:2 * t + 2, ns],
                                         start=(t == 0), stop=(t == 1), perf_mode=DR)
                    if gpb is not None:
                        nc.scalar.activation(ps, ps, AF.Identity,
                                             bias=gpb[:, mo:mo + 1], scale=1.0 / WS)
                        nc.vector.tensor_tensor(xt[:, mo, ns], xt[:, mo, ns], ps, ALU.add)
                    else:
                        nc.vector.scalar_tensor_tensor(xt[:, mo, ns], ps, 1.0 / WS,
                                                       xt[:, mo, ns], ALU.mult, ALU.add)

                # local proj (bf16 moving, fp8 stationary) + residual
                for mo in range(GC):
                    ps = pq_p.tile([128, NHW], F32, tag="pq")
                    for t in range(GC):
                        nc.tensor.matmul(ps, wpl[:, t, mo * 128:(mo + 1) * 128],
                                         o16Tl[:, t, ns],
                                         start=(t == 0), stop=(t == GC - 1))
                    if lpb is not None:
                        nc.scalar.activation(ps, ps, AF.Identity,
                                             bias=lpb[:, mo:mo + 1], scale=1.0 / (WS * WS))
                        nc.vector.tensor_tensor(xt[:, GC + mo, ns], xt[:, GC + mo, ns],
                                                ps, ALU.add)
                    else:
                        nc.vector.scalar_tensor_tensor(xt[:, GC + mo, ns], ps,
                                                       1.0 / (WS * WS),
                                                       xt[:, GC + mo, ns], ALU.mult, ALU.add)
                if end_hook is not None:
                    end_hook(ps_p)

        ln2_ctx = {}

        def emit_ln2_stats(nh, ar_p, ps_pool):
            """LN2 stats on PE (chained sum matmuls into a scores-pool psum
            tile) + DVE smalls.  Emitted inside the scores scope."""
            ns = slice(nh * NHW, (nh + 1) * NHW)
            inv = 1.0 / C
            sq16 = ar_p.tile([128, CC, NHW], BF16, tag="h16")
            if nh == 0:
                nc.scalar.activation(sq16, xt[:, :, ns], AF.Square)
            else:
                nc.vector.tensor_tensor(sq16, xt[:, :, ns], xt[:, :, ns], ALU.mult)
            st = ps_pool.tile([128, 2 * NHW], F32, tag="ps", name="lnst")[0:1, :]
            for i in range(CC):
                nc.tensor.matmul(st[:, 0:NHW], ones_c, xt[:, i, ns],
                                 start=(i == 0), stop=(i == CC - 1))
            for i in range(CC):
                nc.tensor.matmul(st[:, NHW:2 * NHW], ones_c, sq16[:, i, :],
                                 start=(i == 0), stop=(i == CC - 1))
            me = ar_p.tile([1, 2 * NHW], F32, tag="me")
            nc.vector.tensor_scalar_mul(me, st, inv)
            mean, e2 = me[:, 0:NHW], me[:, NHW:2 * NHW]
            m2 = ar_p.tile([1, NHW], F32, tag="m2")
            nc.vector.tensor_tensor(m2, mean, mean, ALU.mult)
            var = ar_p.tile([1, NHW], F32, tag="var")
            nc.vector.tensor_tensor(var, e2, m2, ALU.subtract)
            ln2_ctx[nh] = (var, mean)

        def emit_ln2_tail(nh, ar_p, ps_pool):
            """Sqrt + bf16 broadcasts + 2x tails + h8/hlo8 split."""
            ns = slice(nh * NHW, (nh + 1) * NHW)
            var, mean = ln2_ctx.pop(nh)
            nc.scalar.activation(var, var, AF.Sqrt, bias=eps16_t, scale=1.0 / 16.0)
            rstd = ar_p.tile([1, NHW], BF16, tag="rstd")
            with nc.allow_low_precision(reason="bf16 ln scale"):
                nc.vector.reciprocal(rstd, var)
            mrb = ar_p.tile([1, NHW], BF16, tag="mrb")
            nc.vector.tensor_tensor(mrb, rstd, mean, ALU.mult)
            bc = ps_pool.tile([128, 2 * NHW], F32, tag="ps", name="lnbc")
            nc.tensor.matmul(bc[:, 0:NHW], onebc, rstd, start=True, stop=True)
            nc.tensor.matmul(bc[:, NHW:2 * NHW], onebc, mrb, start=True, stop=True)
            rb16 = ar_p.tile([128, NHW], BF16, tag="rb16")
            nc.vector.tensor_copy(rb16, bc[:, 0:NHW])
            mrb16 = ar_p.tile([128, NHW], BF16, tag="mrb16")
            nc.vector.tensor_copy(mrb16, bc[:, NHW:2 * NHW])
            h16 = ar_p.tile([128, CC, NHW], BF16, tag="h16")
            for c in range(CC):
                t16 = ar_p.tile([128, NHW], BF16, tag="t16")
                nc.vector.tensor_tensor(t16, xt[:, c, ns], rb16, ALU.mult)
                nc.vector.tensor_tensor(h16[:, c, :], t16, mrb16, ALU.subtract)
            for c in range(CC):
                nc.gpsimd.tensor_copy(out=h8[:, c, ns], in_=h16[:, c, :])
                nc.gpsimd.tensor_tensor(hlo8[:, c, ns], h16[:, c, :],
                                        h8[:, c, ns], ALU.subtract)

        def emit_fc1(nh, gl_p, pm_p, jlist, gl8=None):
            ns = slice(nh * NHW, (nh + 1) * NHW)
            if gl8 is None:
                gl8 = gl_p.tile([128, JC, NHW], FP8, tag="gl8", name=f"gl8_{nh}")
            if True:
                for j in jlist:
                    pm = pm_p.tile([128, NHW], F32, tag="pm")
                    chains = [(w1h, h8), (w1h, hlo8)]
                    for ci, (wmat, act) in enumerate(chains):
                        for t in range(GC):
                            nc.tensor.matmul(pm,
                                             wmat[:, 2 * t:2 * t + 2,
                                                  j * 128:(j + 1) * 128],
                                             act[:, 2 * t:2 * t + 2, ns],
                                             start=(ci == 0 and t == 0),
                                             stop=(ci == 1 and t == GC - 1),
                                             perf_mode=DR)
                    nc.scalar.activation(gl8[:, j, :], pm, AF.Gelu,
                                         bias=fc1b[:, j:j + 1], scale=1.0 / (4.0 * WS1))
            return gl8

        def emit_fc2(nh, gl8, out_p, pz_p, mo_list, outT):
            ns = slice(nh * NHW, (nh + 1) * NHW)
            if True:
                for mo in mo_list:
                    zp = pz_p.tile([128, NHW], F32, tag="z")
                    for t2 in range(JC // 2):
                        for wmat, first, last in ((w2h, t2 == 0, False),
                                                  (w2l, False, t2 == JC // 2 - 1)):
                            nc.tensor.matmul(zp,
                                             wmat[:, 2 * t2:2 * t2 + 2,
                                                  mo * 128:(mo + 1) * 128],
                                             gl8[:, 2 * t2:2 * t2 + 2, :],
                                             start=first, stop=last, perf_mode=DR)
                    if fc2b is not None:
                        nc.scalar.activation(zp, zp, AF.Identity,
                                             bias=fc2b[:, mo:mo + 1], scale=1.0 / WS2)
                        nc.vector.tensor_tensor(outT[:, mo, :], xt[:, mo, ns],
                                                zp, ALU.add)
                    else:
                        nc.vector.scalar_tensor_tensor(outT[:, mo, :], zp,
                                                       1.0 / WS2,
                                                       xt[:, mo, ns], ALU.mult, ALU.add)
                    if mo % 2 == 1:
                        nc.sync.dma_start(
                            out_d.rearrange("(c p) n -> p c n", p=128)[:, mo - 1:mo + 1, ns],
                            outT[:, mo - 1:mo + 1, :])
            return outT

        # schedule: scores0 -> [scores1 || LN2-0] -> [MLP0 || LN2-1] -> MLP1
        ar_scope = tc.tile_pool(name="arp", bufs=1)
        ar_p = ar_scope.__enter__()
        emit_scores(0, [0, 1, 2, 3], [0, 0, 1, 1, 1, 1])

        def s1_end(ps):
            emit_ln2_stats(1, ar_p, ps)
            emit_ln2_tail(1, ar_p, ps)

        emit_scores(1, [4, 5, 6, 7], [2, 1, 1, 0, 0, 0],
                    hooks={0: lambda ps: emit_ln2_stats(0, ar_p, ps),
                           3: lambda ps: emit_ln2_tail(0, ar_p, ps)},
                    end_hook=s1_end)
        ar_scope.__exit__(None, None, None)
        ol_scope.__exit__(None, None, None)
        lw_scope.__exit__(None, None, None)
        loc_scope.__exit__(None, None, None)
        with tc.tile_pool(name="gl", bufs=1) as gl_p, \
             tc.tile_pool(name="outp", bufs=1) as out_p, \
             tc.tile_pool(name="pm", bufs=6, space="PSUM") as pm_p, \
             tc.tile_pool(name="pz", bufs=2, space="PSUM") as pz_p:
            gl8_0 = emit_fc1(0, gl_p, pm_p, list(range(JC)))
            outT0 = out_p.tile([128, CC, NHW], F32, tag="outT", name="outT0")
            outT1 = out_p.tile([128, CC, NHW], F32, tag="outT", name="outT1")
            # interleave fc2(nh0) chains with fc1(nh1) chains on the PE queue
            gl8_1 = None
            for mo in range(CC):
                emit_fc2(0, gl8_0, out_p, pz_p, [mo], outT0)
                gl8_1 = emit_fc1(1, gl_p, pm_p, list(range(4 * mo, 4 * mo + 4)),
                                 gl8=gl8_1)
            emit_fc2(1, gl8_1, out_p, pz_p, list(range(CC)), outT1)

        attn_scope.__exit__(None, None, None)

    nc.compile()
    return nc


def _prep_weights(inp):
    """Host-side: fold LN2 affine into fc1, quantize weights to fp8 with
    residual-compensation pairs for the MLP."""
    def q8(w, scale):
        return np.clip(w.astype(np.float64) * scale, -240.0, 240.0).astype(NP_FP8)

    def q8_pair(w, scale):
        hi = q8(w, scale)
        resid = w.astype(np.float64) * scale - hi.astype(np.float64)
        lo = np.clip(resid, -240.0, 240.0).astype(NP_FP8)
        return hi, lo

    def pad_rows(w, rows):
        out = np.zeros((rows, w.shape[1]), np.float64)
        out[:w.shape[0]] = w
        return out

    gqkv = inp["g_qkv_w"]
    lqkv = inp["l_qkv_w"]
    fc1_w = inp["fc1_w"] * inp["ln2_g"][:, None]
    fc1_bias = inp["fc1_b"].astype(np.float64) + \
        inp["ln2_b"].astype(np.float64) @ inp["fc1_w"].astype(np.float64)
    w1h, w1l = q8_pair(fc1_w, WS1)
    w2h, w2l = q8_pair(inp["fc2_w"], WS2)
    return {
        "wqk8": q8(pad_rows(gqkv[:, :2 * GD], 512), WS),
        "wv8": q8(pad_rows(gqkv[:, 2 * GD:], 512), WS),
        "wl8": q8(pad_rows(lqkv, 512), WS),
        "wpg8": q8(pad_rows(inp["g_proj_w"], 512), WS),
        "wpl8": q8(pad_rows(inp["l_proj_w"], 512), WS),
        "w1h8": w1h, "w1l8": w1l, "w2h8": w2h, "w2l8": w2l,
        "fc1bias": fc1_bias.astype(np.float32),
    }


_NC_CACHE = {}


def kernel(**inputs):
    inp = {k: np.ascontiguousarray(np.asarray(v), dtype=np.float32) for k, v in inputs.items()}
    wmap = _prep_weights(inp)
    flags = {
        "gb1g": not (np.all(inp["ln1_g"] == 1.0) and np.all(inp["ln1_b"] == 0.0)),
        "gb1l": not (np.all(inp["ln1l_g"] == 1.0) and np.all(inp["ln1l_b"] == 0.0)),
        "bias_gproj": bool(np.any(inp["g_proj_b"] != 0.0)),
        "bias_lproj": bool(np.any(inp["l_proj_b"] != 0.0)),
        "bias_fc1": bool(np.any(wmap["fc1bias"] != 0.0)),
        "bias_fc2": bool(np.any(inp["fc2_b"] != 0.0)),
    }
    key = tuple(sorted(flags.items()))
    nc = _NC_CACHE.get(key)
    if nc is None:
        nc = _build(flags)
        _NC_CACHE[key] = nc

    if flags["bias_gproj"]:
        wmap["gpb"] = inp["g_proj_b"]
    if flags["bias_lproj"]:
        wmap["lpb"] = inp["l_proj_b"]
    if flags["bias_fc2"]:
        wmap["fc2bias"] = inp["fc2_b"]
    if flags["gb1g"]:
        wmap["ln1gb"] = np.stack([inp["ln1_g"], inp["ln1_b"]])
    if flags["gb1l"]:
        wmap["ln1lgb"] = np.stack([inp["ln1l_g"], inp["ln1l_b"]])

    x = inp["x"]
    in_maps = [dict(wmap, xT=np.ascontiguousarray(x[b].T).astype(NP_BF16))
               for b in range(B)]
    trace = os.environ.get("BASS_KERNEL_TRACE", "") == "1"
    res = run_bass_kernel_spmd(nc, in_maps, core_ids=list(range(B)),
                               trace=trace, trace_cores=[0] if trace else None)
    if trace:
        print(f"HW exec time: {res.exec_time_ns} ns")
        if res.instructions_and_trace:
            print("trace path:", res.instructions_and_trace[1])
    return np.stack([np.asarray(res.results[b]["outT"]).astype(np.float32).T
                     for b in range(B)])
